# revision 1
# baseline (speedup 1.0000x reference)
"""Trainium2 Bass kernel for nn_BranchGCN (gnn_message_passing).

Strategy (8 NeuronCores, two SPMD launches):
  Stage A -- model-parallel over W_branch's node axis: core c owns nodes
    [4c, 4c+4). It computes root aggregation + per-node branch matmul +
    loop MLP for its 4 nodes x all 16 samples, and emits x (point coords)
    plus |x|^2 for its 256 of the 2048 graph rows.  This reads only 1/8 of
    the 128 MiB W_branch per core (the dominant memory term).
  Host    -- reshards (pure concatenation/transpose, no arithmetic).
  Stage B -- row-sharded EdgeConv: every core holds all 2048 points of all
    16 samples (small), computes the KNN top-8 for its 256 rows x 16
    samples via PE distance matmuls + DVE max8/max_index, gathers the
    factored conv values via indirect DMA, and applies max/bias/leaky.

EdgeConv factorization used (exact, modulo fp reassociation):
  h2[b,n,k,:] = (feat-center) @ M1 + center @ M2 + (c1b @ c2w.T + c2b)
    with M1 = c1w[:, :3].T @ c2w.T,  M2 = c1w[:, 3:].T @ c2w.T
  out_pre[n]  = max_k (x[idx_k] @ M1)  +  x[n] @ (M2 - M1) + const
  pd[n,j] is computed with the 5-term homogeneous matmul
    [2x_n, -|x_n|^2, -1] . [x_j, 1, |x_j|^2]  so pd[n,n] == 0 exactly and
  the self row is always rank-0 of the top-8, which lets the same gather
  fetch the center term (z) from the table's columns 3:6 at k=0.
"""

import os
import sys
import numpy as np

sys.path.insert(0, "/opt/trn_rl_repo")

from contextlib import ExitStack

import concourse.tile as tile
from concourse import bacc, bass, mybir
from concourse.bass import AP
from concourse.bass_utils import run_bass_kernel_spmd
from concourse.masks import make_identity

FP = mybir.dt.float32
U32 = mybir.dt.uint32

B, NODE, DEG, K = 16, 32, 64, 8
IN_F, OUT_F, SUP = 128, 3, 10
FEATS = [96, 256, 256, 256, 128, 128]
SIZES = [1, 2, 4, 8, 16, 32]
NCORES = 8
NLOC = NODE // NCORES          # 4 nodes per core
N = NODE * DEG                 # 2048 graph rows
RLOC = NLOC * DEG              # 256 rows per core
ALU = mybir.AluOpType
AF = mybir.ActivationFunctionType


# --------------------------------------------------------------------------
# Stage A program: branch/root/loop-MLP for this core's 4 nodes.
# --------------------------------------------------------------------------
def build_stage_a():
    nc = bacc.Bacc(None)
    tls = [nc.declare_dram_parameter(f"tl{i}", [B, NLOC, FEATS[i]], FP, isOutput=False)
           for i in range(6)]
    wrs = [nc.declare_dram_parameter(f"wr{i}", [FEATS[i], OUT_F], FP, isOutput=False)
           for i in range(6)]
    wb = nc.declare_dram_parameter("wb", [NLOC, IN_F, DEG * IN_F], FP, isOutput=False)
    wl1 = nc.declare_dram_parameter("wl1", [IN_F, IN_F * SUP], FP, isOutput=False)
    wl2 = nc.declare_dram_parameter("wl2", [IN_F * SUP, OUT_F], FP, isOutput=False)
    # rows 0-2: x coords, row 3: |x|^2 ; flat order = (b, n_local, d)
    xchunk = nc.declare_dram_parameter("xchunk", [4, B * RLOC], FP,
                                       isOutput=True)

    with tile.TileContext(nc) as tc, ExitStack() as ctx:
        sbp = ctx.enter_context(tc.tile_pool(name="sbuf", bufs=1))
        wbpool = ctx.enter_context(tc.tile_pool(name="wbuf", bufs=2))
        psp = ctx.enter_context(tc.tile_pool(name="psum", bufs=1,
                                             space="PSUM"))
        pbp = ctx.enter_context(tc.tile_pool(name="psumb", bufs=2,
                                             space="PSUM"))

        ident = sbp.tile([128, 128], FP)
        make_identity(nc, ident[:])

        # ---- load + transpose the per-node tree slices: tlT[i] = (f, 64)
        tlT = []
        for i in range(6):
            f = FEATS[i]
            nat = sbp.tile([B * NLOC, f], FP, tag=f"tlnat{i}")
            nc.sync.dma_start(out=nat[:],
                              in_=tls[i][:].rearrange("b n f -> (b n) f"))
            nchunk = (f + 127) // 128
            tt = sbp.tile([128, nchunk, B * NLOC], FP, tag=f"tlT{i}")
            for c in range(nchunk):
                cw = min(128, f - c * 128)
                pt = psp.tile([128, B * NLOC], FP, tag="ptr")
                nc.tensor.transpose(out=pt[:cw, :],
                                    in_=nat[:, c * 128:c * 128 + cw],
                                    identity=ident[0:B * NLOC, 0:B * NLOC])
                nc.scalar.activation(out=tt[0:cw, c, :], in_=pt[:cw, :],
                                     func=AF.Copy)
            tlT.append(tt)

        # ---- Wl = Wl1 @ Wl2  (128, 3)
        wl1_sb = sbp.tile([128, IN_F * SUP], FP)
        nc.sync.dma_start(out=wl1_sb[:], in_=wl1[:])
        wl2_sb = sbp.tile([128, SUP, OUT_F], FP)
        nc.sync.dma_start(out=wl2_sb[:],
                          in_=wl2[:].rearrange("(c p) o -> p c o", p=128))
        wl1T = sbp.tile([128, SUP, 128], FP)
        for c in range(SUP):
            pt = psp.tile([128, 128], FP, tag="ptw")
            nc.tensor.transpose(out=pt[:], in_=wl1_sb[:, c * 128:(c + 1) * 128],
                                identity=ident[:])
            nc.scalar.activation(out=wl1T[:, c, :], in_=pt[:], func=AF.Copy)
        pwl = psp.tile([128, OUT_F], FP, tag="pwl")
        for c in range(SUP):
            nc.tensor.matmul(out=pwl[:], lhsT=wl1T[:, c, :],
                             rhs=wl2_sb[:, c, :],
                             start=(c == 0), stop=(c == SUP - 1))
        wlv = sbp.tile([128, OUT_F], FP)
        nc.scalar.activation(out=wlv[:], in_=pwl[:], func=AF.Copy)

        # ---- root aggregation for this core's nodes: rootT (3, (b, nl))
        wr_sb = []
        for i in range(6):
            f = FEATS[i]
            nchunk = (f + 127) // 128
            w = sbp.tile([128, nchunk, OUT_F], FP, tag=f"wr{i}")
            nc.sync.dma_start(
                out=w[:f if nchunk == 1 else 128, :, :],
                in_=wrs[i][:].rearrange("(c p) o -> p c o",
                                        c=nchunk) if nchunk > 1
                else wrs[i][:].unsqueeze(1))
            wr_sb.append(w)
        proot = psp.tile([OUT_F, B * NLOC], FP, tag="proot")
        steps = []
        for i in range(6):
            for c in range((FEATS[i] + 127) // 128):
                steps.append((i, c))
        for si, (i, c) in enumerate(steps):
            f = FEATS[i]
            cw = min(128, f - c * 128)
            nc.tensor.matmul(out=proot[:],
                             lhsT=wr_sb[i][:cw, c, :],
                             rhs=tlT[i][0:cw, c, :],
                             start=(si == 0), stop=(si == len(steps) - 1))
        rootT = sbp.tile([OUT_F, B * NLOC], FP)
        nc.scalar.activation(out=rootT[:], in_=proot[:], func=AF.Copy)

        # ---- branch einsum + leaky: branchT (128, (b, nl, d))
        branchT = sbp.tile([128, B * RLOC], FP)
        t5v = tlT[5][:, 0, :].rearrange("p (b n) -> p n b", n=NLOC)
        for nl in range(NLOC):
            wbt = wbpool.tile([128, DEG * IN_F], FP, tag="wbt")
            nc.sync.dma_start(out=wbt[:], in_=wb[nl])
            for g in range(2):
                pb = pbp.tile([128, 512], FP, tag="pbranch")
                for dl in range(32):
                    d = g * 32 + dl
                    nc.tensor.matmul(out=pb[:, dl * 16:(dl + 1) * 16],
                                     lhsT=wbt[:, d * 128:(d + 1) * 128],
                                     rhs=t5v[:, nl, :],
                                     start=True, stop=True)
                # out view: (p, dl, b) -> branchT[(b, nl, d=g*32+dl)]
                ov = branchT[:].rearrange(
                    "p (b n g dl) -> p n g dl b", n=NLOC, g=2, dl=32)
                pbs = sbp.tile([128, 512], FP, tag="pbs")
                nc.scalar.activation(out=pbs[:], in_=pb[:], func=AF.Copy)
                pbv = pbs[:].rearrange("p (dl b) -> p dl b", dl=32)
                nc.vector.scalar_tensor_tensor(
                    out=ov[:, nl, g, :, :], in0=pbv, scalar=0.2, in1=pbv,
                    op0=ALU.mult, op1=ALU.max)

        # ---- x = branch @ Wl + root(repeat d); then |x|^2
        x3 = sbp.tile([OUT_F, B * RLOC], FP)
        xx1 = sbp.tile([1, B * RLOC], FP)
        rootv = rootT[:].rearrange("p (b n) -> p b n", n=NLOC)
        for ch in range(8):  # 512 cols = 2 samples each
            po = psp.tile([OUT_F, 512], FP, tag="po3")
            nc.tensor.matmul(out=po[:], lhsT=wlv[:, :OUT_F],
                             rhs=branchT[:, ch * 512:(ch + 1) * 512],
                             start=True, stop=True)
            in0 = po[:].rearrange("p (b n d) -> p b n d", b=2, n=NLOC)
            in1 = rootv[:, 2 * ch:2 * ch + 2, :].unsqueeze(3).to_broadcast(
                [OUT_F, 2, NLOC, DEG])
            ov = x3[:].rearrange("p (b n d) -> p b n d", b=B,
                                 n=NLOC)[:, 2 * ch:2 * ch + 2]
            nc.vector.tensor_tensor(out=ov, in0=in0, in1=in1, op=ALU.add)
        sq = sbp.tile([OUT_F, B * RLOC], FP)
        nc.vector.tensor_tensor(out=sq[:], in0=x3[:], in1=x3[:],
                                op=ALU.mult)
        ones3 = sbp.tile([OUT_F, 1], FP)
        nc.vector.memset(ones3[:], 1.0)
        for ch in range(8):
            px = psp.tile([1, 512], FP, tag="pxx")
            nc.tensor.matmul(out=px[:], lhsT=ones3[:],
                             rhs=sq[:, ch * 512:(ch + 1) * 512],
                             start=True, stop=True)
            nc.scalar.activation(out=xx1[0:1, ch * 512:(ch + 1) * 512],
                                 in_=px[:], func=AF.Copy)
        nc.sync.dma_start(out=xchunk[0:3, :], in_=x3[:])
        nc.sync.dma_start(out=xchunk[3:4, :], in_=xx1[:])
    return nc


# --------------------------------------------------------------------------
# Stage B program: KNN EdgeConv for this core's 256 rows x 16 samples.
# --------------------------------------------------------------------------
def build_stage_b():
    nc = bacc.Bacc(None)
    # rows: [x0, x1, x2, xx]; per-sample row-major (node*64+d)
    vall = nc.declare_dram_parameter("vall", [4, B, N], FP, isOutput=False)
    # rows: [x0, x1, x2] for this core's 256 rows, flat (b, nl, d)
    urx = nc.declare_dram_parameter("urx", [OUT_F, B * RLOC], FP, isOutput=False)
    biasd = nc.declare_dram_parameter("biasd", [DEG, OUT_F], FP, isOutput=False)
    c1w = nc.declare_dram_parameter("c1w", [64, 6], FP, isOutput=False)
    c1b = nc.declare_dram_parameter("c1b", [64, 1], FP, isOutput=False)
    c2w = nc.declare_dram_parameter("c2w", [OUT_F, 64], FP, isOutput=False)
    c2b = nc.declare_dram_parameter("c2b", [1, OUT_F], FP, isOutput=False)
    outc = nc.declare_dram_parameter("outc", [B, RLOC, OUT_F], FP,
                                     isOutput=True)

    with tile.TileContext(nc) as tc, ExitStack() as ctx:
        sbp = ctx.enter_context(tc.tile_pool(name="sbuf", bufs=1))
        lop = ctx.enter_context(tc.tile_pool(name="loop", bufs=2))
        dramp = ctx.enter_context(tc.tile_pool(name="dram", bufs=1,
                                               space="DRAM"))
        psw = ctx.enter_context(tc.tile_pool(name="psw", bufs=2,
                                             space="PSUM"))
        pst = ctx.enter_context(tc.tile_pool(name="pst", bufs=2,
                                             space="PSUM"))
        setup_ps = tc.tile_pool(name="setup_ps", bufs=2, space="PSUM")
        psu = setup_ps.__enter__()
        setup_sb = tc.tile_pool(name="setup_sb", bufs=1)
        sbu = setup_sb.__enter__()

        ident = sbp.tile([128, 128], FP)
        make_identity(nc, ident[:])

        # ---- conv weight factorization on device
        c1w_sb = sbu.tile([64, 6], FP)
        nc.sync.dma_start(out=c1w_sb[:], in_=c1w[:])
        c1b_sb = sbu.tile([64, 1], FP)
        nc.sync.dma_start(out=c1b_sb[:], in_=c1b[:])
        c2w_sb = sbu.tile([OUT_F, 64], FP)
        nc.sync.dma_start(out=c2w_sb[:], in_=c2w[:])
        c2b_sb = sbu.tile([1, OUT_F], FP)
        nc.sync.dma_start(out=c2b_sb[:], in_=c2b[:])

        ptc = psu.tile([64, OUT_F], FP, tag="su")
        nc.tensor.transpose(out=ptc[:], in_=c2w_sb[:],
                            identity=ident[0:OUT_F, 0:OUT_F])
        c2wT = sbu.tile([64, OUT_F], FP)
        nc.scalar.activation(out=c2wT[:], in_=ptc[:], func=AF.Copy)
        c2wTn = sbu.tile([64, OUT_F], FP)
        nc.vector.tensor_scalar_mul(out=c2wTn[:], in0=c2wT[:], scalar1=-1.0)

        lw = sbp.tile([OUT_F, 8], FP)       # lhsT for the w-table matmul
        nc.vector.memset(lw[:], 0.0)
        pm1 = psu.tile([OUT_F, OUT_F], FP, tag="su")
        nc.tensor.matmul(out=pm1[:], lhsT=c1w_sb[:, 0:3], rhs=c2wT[:],
                         start=True, stop=True)
        nc.vector.tensor_copy(out=lw[0:3, 0:3], in_=pm1[:])
        pm2 = psu.tile([OUT_F, OUT_F], FP, tag="su")
        nc.tensor.matmul(out=pm2[:], lhsT=c1w_sb[:, 3:6], rhs=c2wT[:],
                         start=True, stop=False)
        nc.tensor.matmul(out=pm2[:], lhsT=c1w_sb[:, 0:3], rhs=c2wTn[:],
                         start=False, stop=True)
        nc.vector.tensor_copy(out=lw[0:3, 3:6], in_=pm2[:])

        # zc = c1b @ c2w.T + c2b, broadcast to 128 partitions, fold into bias
        pzc = psu.tile([1, OUT_F], FP, tag="su")
        nc.tensor.matmul(out=pzc[:], lhsT=c1b_sb[:], rhs=c2wT[:],
                         start=True, stop=True)
        zrow = sbu.tile([1, OUT_F], FP)
        nc.vector.tensor_tensor(out=zrow[:], in0=pzc[:], in1=c2b_sb[:],
                                op=ALU.add)
        ones1 = sbu.tile([1, 128], FP)
        nc.vector.memset(ones1[:], 1.0)
        pzb = psu.tile([128, OUT_F], FP, tag="su")
        nc.tensor.matmul(out=pzb[:], lhsT=ones1[:], rhs=zrow[:],
                         start=True, stop=True)
        bias_sb = sbu.tile([128, OUT_F], FP)
        nc.sync.dma_start(out=bias_sb[0:64, :], in_=biasd[:])
        nc.sync.dma_start(out=bias_sb[64:128, :], in_=biasd[:])
        bias2 = sbp.tile([128, OUT_F], FP)
        nc.vector.tensor_tensor(out=bias2[:], in0=bias_sb[:], in1=pzb[:],
                                op=ALU.add)

        # ---- static tiles
        vall_sb = sbp.tile([4, B, N], FP)
        nc.sync.dma_start(out=vall_sb[:], in_=vall[:])
        uv = sbp.tile([4, B * RLOC], FP)
        nc.sync.dma_start(out=uv[0:3, :], in_=urx[:])
        nc.vector.tensor_scalar_mul(out=uv[0:3, :], in0=uv[0:3, :],
                                    scalar1=2.0)
        mone = sbu.tile([1, B * RLOC], FP)
        nc.vector.memset(mone[:], -1.0)
        nc.sync.dma_start(out=uv[3:4, :], in_=mone[:])
        setup_sb.__exit__(None, None, None)
        setup_ps.__exit__(None, None, None)
        pspd = ctx.enter_context(tc.tile_pool(name="pspd", bufs=1,
                                              space="PSUM"))

        final_sb = sbp.tile([128, B, 2, OUT_F], FP)

        for b in range(B):
            # ---- w table: w[row] = [y(3), z(3), 0, 0]
            wT_sb = lop.tile([8, N], FP, tag="wT")
            for chm in range(4):
                pw = psw.tile([8, 512], FP, tag="pw")
                nc.tensor.matmul(out=pw[:], lhsT=lw[:],
                                 rhs=vall_sb[0:3, b, chm * 512:(chm + 1) * 512],
                                 start=True, stop=True)
                nc.scalar.activation(out=wT_sb[:, chm * 512:(chm + 1) * 512],
                                     in_=pw[:], func=AF.Copy)
            ptr = pst.tile([128, 128], FP, tag="ptr")
            for kk in range(16):
                nc.tensor.transpose(out=ptr[:, kk * 8:(kk + 1) * 8],
                                    in_=wT_sb[:, kk * 128:(kk + 1) * 128],
                                    identity=ident[0:8, 0:8])
            wrows = lop.tile([128, 16 * 8], FP, tag="wrows")
            nc.scalar.activation(out=wrows[:], in_=ptr[:], func=AF.Copy)
            wtab = dramp.tile([N, 8], FP, tag=f"wtab{b}")
            nc.sync.dma_start(
                out=wtab[:].rearrange("(k p) e -> p k e", p=128),
                in_=wrows[:].rearrange("p (k e) -> p k e", k=16))

            # ---- pd + top8 for the two 128-row tiles
            idx = lop.tile([128, 2, K], U32, tag="idx")
            for m in range(2):
                ppd = pspd.tile([128, N], FP, tag="ppd")
                for chm in range(4):
                    nc.tensor.matmul(
                        out=ppd[:, chm * 512:(chm + 1) * 512],
                        lhsT=uv[:, b * RLOC + m * 128:b * RLOC + (m + 1) * 128],
                        rhs=vall_sb[:, b, chm * 512:(chm + 1) * 512],
                        start=True, stop=True)
                pd_sb = lop.tile([128, N], FP, tag="pd")
                nc.scalar.activation(out=pd_sb[:], in_=ppd[:], func=AF.Copy)
                top8 = lop.tile([128, K], FP, tag="top8")
                nc.vector.max(out=top8[:], in_=pd_sb[:])
                nc.vector.max_index(out=idx[:, m, :], in_max=top8[:],
                                    in_values=pd_sb[:])

            # ---- gather w[idx] : (128, 2, 8, 8)
            gth = lop.tile([128, 2, K, 8], FP, tag="gth")
            for m in range(2):
                for k in range(K):
                    nc.gpsimd.indirect_dma_start(
                        out=gth[:, m, k, :],
                        out_offset=None,
                        in_=wtab[:],
                        in_offset=bass.IndirectOffsetOnAxis(
                            ap=idx[:, m, k:k + 1], axis=0),
                    )

            # ---- max over neighbors + center term + bias + leaky
            red = lop.tile([128, 2, OUT_F], FP, tag="red")
            nc.vector.tensor_reduce(
                out=red[:],
                in_=gth[:].rearrange("p m k e -> p m e k")[:, :, 0:OUT_F, :],
                axis=mybir.AxisListType.X, op=ALU.max)
            tmp = lop.tile([128, 2, OUT_F], FP, tag="tmp")
            nc.vector.tensor_tensor(out=tmp[:], in0=red[:],
                                    in1=gth[:, :, 0, 3:3 + OUT_F], op=ALU.add)
            nc.vector.tensor_tensor(
                out=tmp[:], in0=tmp[:],
                in1=bias2[:].unsqueeze(1).to_broadcast([128, 2, OUT_F]),
                op=ALU.add)
            nc.vector.scalar_tensor_tensor(
                out=final_sb[:, b, :, :], in0=tmp[:], scalar=0.2, in1=tmp[:],
                op0=ALU.mult, op1=ALU.max)

        nc.sync.dma_start(
            out=outc[:].rearrange("b (m p) o -> p b m o", p=128),
            in_=final_sb[:])
    return nc


# --------------------------------------------------------------------------
# Host orchestration
# --------------------------------------------------------------------------
_CACHE = {}
LAST_RESULTS = {}


def _programs():
    if "a" not in _CACHE:
        nca = build_stage_a()
        nca.compile()
        ncb = build_stage_b()
        ncb.compile()
        _CACHE["a"] = nca
        _CACHE["b"] = ncb
    return _CACHE["a"], _CACHE["b"]


def _stage_a_inmaps(inputs):
    trees = [np.asarray(inputs[f"t{i}"], np.float32) for i in range(6)]
    wrs = [np.asarray(inputs[f"Wr{i}"], np.float32) for i in range(6)]
    wb = np.asarray(inputs["W_branch"], np.float32)
    in_maps = []
    for c in range(NCORES):
        m = {}
        nodes = [4 * c + j for j in range(NLOC)]
        for i in range(6):
            rows = [n * SIZES[i] // NODE for n in nodes]
            m[f"tl{i}"] = np.ascontiguousarray(trees[i][:, rows, :])
            m[f"wr{i}"] = wrs[i]
        m["wb"] = np.ascontiguousarray(wb[nodes])
        m["wl1"] = np.asarray(inputs["Wl1"], np.float32)
        m["wl2"] = np.asarray(inputs["Wl2"], np.float32)
        in_maps.append(m)
    return in_maps


def _stage_b_inmaps(inputs, xchunks):
    # xchunks: list of (4, B*RLOC) arrays, flat order (b, nl, d)
    xs = np.stack(xchunks)                     # (8, 4, B*256)
    xs = xs.reshape(NCORES, 4, B, RLOC)        # (c, comp, b, nl*64+d)
    vall = np.ascontiguousarray(
        xs.transpose(1, 2, 0, 3).reshape(4, B, N))   # rows node*64+d
    bias = np.asarray(inputs["bias"], np.float32).reshape(DEG, OUT_F)
    in_maps = []
    for c in range(NCORES):
        m = {
            "vall": vall,
            "urx": np.ascontiguousarray(xs[c, 0:3].reshape(OUT_F, B * RLOC)),
            "biasd": bias,
            "c1w": np.asarray(inputs["c1w"], np.float32),
            "c1b": np.asarray(inputs["c1b"], np.float32).reshape(64, 1),
            "c2w": np.asarray(inputs["c2w"], np.float32),
            "c2b": np.asarray(inputs["c2b"], np.float32).reshape(1, OUT_F),
        }
        in_maps.append(m)
    return in_maps


def kernel(**inputs):
    nca, ncb = _programs()
    core_ids = list(range(NCORES))

    ra = run_bass_kernel_spmd(nca, _stage_a_inmaps(inputs), core_ids)
    LAST_RESULTS["a"] = ra
    xchunks = [np.asarray(ra.results[c]["xchunk"]) for c in range(NCORES)]

    rb = run_bass_kernel_spmd(ncb, _stage_b_inmaps(inputs, xchunks), core_ids)
    LAST_RESULTS["b"] = rb
    out = np.empty((B, N, OUT_F), np.float32)
    for c in range(NCORES):
        out[:, c * RLOC:(c + 1) * RLOC, :] = rb.results[c]["outc"]
    return out



# revision 25
# speedup vs baseline: 1.6596x; 1.6596x over previous
"""Trainium2 Bass kernel for nn_BranchGCN (gnn_message_passing).

Two SPMD launches over 8 cores:
  Stage A -- node-model-parallel: core c owns nodes [4c, 4c+4) and streams its
    1/8 slice of W_branch (fp16) while computing per point: x, xx2=|x|^2/2,
    y = M1^T x, u = (M2-M1)^T x + bias + zc, plus an error-compensated fp16
    split (xh + xl) of x and xx2 for stage B's distance matmul. Weight-only
    folds (Wl1@Wl2, conv factorization M1/M2/zc) are host precomputed; all
    per-sample math stays on device.
  Host    -- pure relayout (numpy transposes / dtype casts / constant fills).
  Stage B -- row-sharded KNN EdgeConv. Per 128-row tile, one K=12 fp16 matmul
    (split-compensated, fp32-grade accuracy) yields
      pd[n,j] = x_n.x_j - |x_j|^2/2 - |x_n|^2/2  (~ -|x_n-x_j|^2/2, row-shift)
    Exact top-8 via two pairwise-max folds (2048->512 slots), max8+max_index
    on the folded array, then 4-way slot expansion: the true top-8 provably
    lies among the 32 expanded candidates. One multi-offset indirect DMA
    gathers their [x, xx2, y] rows, pd is recomputed in fp32 for the 32,
    thresholded at the 8th largest, and max_k y + center term u -> leaky.
"""

import sys
import numpy as np

sys.path.insert(0, "/opt/trn_rl_repo")

from contextlib import ExitStack

import concourse.tile as tile
from concourse import bacc, bass, mybir
from concourse.bass_utils import run_bass_kernel_spmd

FP = mybir.dt.float32
FP16 = mybir.dt.float16
U32 = mybir.dt.uint32
ALU = mybir.AluOpType
AF = mybir.ActivationFunctionType

B, NODE, DEG, K = 16, 32, 64, 8
IN_F, OUT_F, SUP = 128, 3, 10
FEATS = [96, 256, 256, 256, 128, 128]
SIZES = [1, 2, 4, 8, 16, 32]
NCORES = 8
NLOC = NODE // NCORES          # 4 nodes per core
N = NODE * DEG                 # 2048 graph rows
RLOC = NLOC * DEG              # 256 rows per core
NP_CORE = B * RLOC             # 4096 points per core
NF = 512                       # fold slots (2048 / 4)
BIG = 60000.0
NCH = [(f + 127) // 128 for f in FEATS]


# --------------------------------------------------------------------------
# Stage A
# --------------------------------------------------------------------------
def build_stage_a():
    nc = bacc.Bacc(None)
    tlT = [nc.declare_dram_parameter(f"tlT{i}", [128, NCH[i] * NLOC * B], FP,
                                     isOutput=False) for i in range(6)]
    wrs = [nc.declare_dram_parameter(f"wr{i}", [128, NCH[i] * OUT_F], FP,
                                     isOutput=False) for i in range(6)]
    wb = nc.declare_dram_parameter("wb", [NLOC, IN_F, DEG * IN_F], FP16,
                                   isOutput=False)
    t5h = nc.declare_dram_parameter("t5h", [IN_F, NLOC, B], FP16,
                                    isOutput=False)
    wbig = nc.declare_dram_parameter("wbig", [IN_F, 16], FP16, isOutput=False)
    abig = nc.declare_dram_parameter("abig", [OUT_F, 16], FP16, isOutput=False)
    bsel = nc.declare_dram_parameter("bsel", [OUT_F, 16], FP16, isOutput=False)
    biasu = nc.declare_dram_parameter("biasu", [OUT_F, DEG], FP16,
                                      isOutput=False)
    # fp32 rows: 0-2 x, 3 xx2, 4-6 y, 8-10 u; cols (nl, b, d)
    xout = nc.declare_dram_parameter("xout", [16, NP_CORE], FP, isOutput=True)
    # fp16 rows: 0-2 xh, 3 xxh2, 4-6 xl, 7 xxl2
    xout16 = nc.declare_dram_parameter("xout16", [8, NP_CORE], FP16,
                                       isOutput=True)

    with tile.TileContext(nc) as tc, ExitStack() as ctx:
        sbp = ctx.enter_context(tc.tile_pool(name="sbuf", bufs=1))
        wbpool = ctx.enter_context(tc.tile_pool(name="wbuf", bufs=2))
        psp = ctx.enter_context(tc.tile_pool(name="psum", bufs=2,
                                             space="PSUM"))
        pbp = ctx.enter_context(tc.tile_pool(name="psumb", bufs=2,
                                             space="PSUM"))
        pxp = ctx.enter_context(tc.tile_pool(name="psumx", bufs=2,
                                             space="PSUM"))

        # ---- small loads
        tl_sb, wr_sb = [], []
        for i in range(6):
            t = sbp.tile([128, NCH[i], NLOC * B], FP, tag=f"tlT{i}")
            nc.sync.dma_start(out=t[:], in_=tlT[i][:].rearrange(
                "p (c n) -> p c n", c=NCH[i]))
            tl_sb.append(t)
            w = sbp.tile([128, NCH[i], OUT_F], FP, tag=f"wr{i}")
            nc.sync.dma_start(out=w[:], in_=wrs[i][:].rearrange(
                "p (c o) -> p c o", c=NCH[i]))
            wr_sb.append(w)
        t5_sb = sbp.tile([IN_F, NLOC, B], FP16)
        nc.sync.dma_start(out=t5_sb[:], in_=t5h[:])
        wbig_sb = sbp.tile([IN_F, 16], FP16)
        nc.sync.dma_start(out=wbig_sb[:], in_=wbig[:])
        abig_sb = sbp.tile([OUT_F, 16], FP16)
        nc.sync.dma_start(out=abig_sb[:], in_=abig[:])
        bsel_sb = sbp.tile([OUT_F, 16], FP16)
        nc.sync.dma_start(out=bsel_sb[:], in_=bsel[:])
        biasu_sb = sbp.tile([OUT_F, DEG], FP16)
        nc.sync.dma_start(out=biasu_sb[:], in_=biasu[:])
        oneh = sbp.tile([OUT_F, 1], FP)
        nc.vector.memset(oneh[:], 0.5)

        # ---- root aggregation: rootT fp16 (3, nl, b)
        proot = psp.tile([OUT_F, NLOC * B], FP, tag="proot")
        steps = [(i, c) for i in range(6) for c in range(NCH[i])]
        for si, (i, c) in enumerate(steps):
            nc.tensor.matmul(out=proot[:],
                             lhsT=wr_sb[i][:, c, :],
                             rhs=tl_sb[i][:, c, :],
                             start=(si == 0), stop=(si == len(steps) - 1))
        rootT = sbp.tile([OUT_F, NLOC, B], FP16)
        nc.scalar.activation(out=rootT[:],
                             in_=proot[:].rearrange("p (nl b) -> p nl b",
                                                    nl=NLOC),
                             func=AF.Copy)

        # ---- per-node: branch einsum (fp16) + leaky -> branchT fp16
        branchT = sbp.tile([IN_F, NLOC, B, 2, 32], FP16)
        xout_sb = sbp.tile([16, NLOC, B, DEG], FP)
        xx2_sb = sbp.tile([1, NLOC, B, DEG], FP)
        xh3_sb = sbp.tile([3, NLOC, B, DEG], FP16)
        xxh2_sb = sbp.tile([1, NLOC, B, DEG], FP16)
        xl3_sb = sbp.tile([3, NLOC, B, DEG], FP16)
        xxl2_sb = sbp.tile([1, NLOC, B, DEG], FP16)
        for nl in range(NLOC):
            wbt = wbpool.tile([IN_F, DEG * IN_F], FP16, tag="wbt")
            nc.sync.dma_start(out=wbt[:], in_=wb[nl])
            for g in range(2):
                pb = pbp.tile([IN_F, 32, B], FP, tag="pbranch")
                for dl in range(32):
                    d = g * 32 + dl
                    nc.tensor.matmul(out=pb[:, dl, :],
                                     lhsT=wbt[:, d * 128:(d + 1) * 128],
                                     rhs=t5_sb[:, nl, :],
                                     start=True, stop=True)
                # transpose-copy PSUM -> SBUF fp16 (b-major), then leaky
                cg = sbp.tile([IN_F, B, 32], FP16, tag="cg")
                nc.vector.tensor_copy(
                    out=cg[:].rearrange("p b dl -> p dl b"), in_=pb[:])
                nc.vector.scalar_tensor_tensor(
                    out=branchT[:, nl, :, g, :], in0=cg[:], scalar=0.2,
                    in1=cg[:], op0=ALU.mult, op1=ALU.max)
            # rows [x, y, u] for this node's points, 2 chunks of (8b x 64d)
            for h in range(2):
                pxo = pxp.tile([16, 8, DEG], FP, tag="pxo")
                nc.tensor.matmul(
                    out=pxo[:],
                    lhsT=wbig_sb[:],
                    rhs=branchT[:, nl, 8 * h:8 * h + 8, :, :].rearrange(
                        "p b g dl -> p (b g dl)"),
                    start=True, stop=False)
                nc.tensor.matmul(
                    out=pxo[:],
                    lhsT=abig_sb[:],
                    rhs=rootT[:, nl, 8 * h:8 * h + 8].unsqueeze(2)
                        .to_broadcast([OUT_F, 8, DEG]),
                    start=False, stop=False)
                nc.tensor.matmul(
                    out=pxo[:],
                    lhsT=bsel_sb[:],
                    rhs=biasu_sb[:].unsqueeze(1)
                        .to_broadcast([OUT_F, 8, DEG]),
                    start=False, stop=True)
                xo = xout_sb[:, nl, 8 * h:8 * h + 8, :]
                nc.scalar.activation(out=xo, in_=pxo[:], func=AF.Copy)
                # xx2 = 0.5 * sum x_c^2
                sq = sbp.tile([OUT_F, 8 * DEG], FP, tag="sq")
                nc.vector.scalar_tensor_tensor(
                    out=sq[:],
                    in0=xo[0:3].rearrange("p b d -> p (b d)"), scalar=0.0,
                    in1=xo[0:3].rearrange("p b d -> p (b d)"),
                    op0=ALU.bypass, op1=ALU.mult)
                pxx = psp.tile([1, 8 * DEG], FP, tag="pxx")
                nc.tensor.matmul(out=pxx[:], lhsT=oneh[:], rhs=sq[:],
                                 start=True, stop=True)
                nc.vector.tensor_copy(
                    out=xx2_sb[:, nl, 8 * h:8 * h + 8, :].rearrange(
                        "p b d -> p (b d)"), in_=pxx[:])
            # fp16 split of x and xx2 for this node (partition-0 aligned ops)
            nc.scalar.activation(out=xh3_sb[:, nl], in_=xout_sb[0:3, nl],
                                 func=AF.Copy)
            nc.scalar.activation(out=xxh2_sb[:, nl], in_=xx2_sb[:, nl],
                                 func=AF.Copy)
            nc.vector.scalar_tensor_tensor(
                out=xl3_sb[:, nl], in0=xout_sb[0:3, nl], scalar=0.0,
                in1=xh3_sb[:, nl], op0=ALU.bypass, op1=ALU.subtract)
            nc.vector.scalar_tensor_tensor(
                out=xxl2_sb[:, nl], in0=xx2_sb[:, nl], scalar=0.0,
                in1=xxh2_sb[:, nl], op0=ALU.bypass, op1=ALU.subtract)

        nc.sync.dma_start(
            out=xout[0:3, :],
            in_=xout_sb[0:3].rearrange("p nl b d -> p (nl b d)"))
        nc.sync.dma_start(
            out=xout[4:16, :],
            in_=xout_sb[4:16].rearrange("p nl b d -> p (nl b d)"))
        nc.sync.dma_start(
            out=xout[3:4, :],
            in_=xx2_sb[:].rearrange("p nl b d -> p (nl b d)"))
        nc.sync.dma_start(
            out=xout16[0:3, :],
            in_=xh3_sb[:].rearrange("p nl b d -> p (nl b d)"))
        nc.sync.dma_start(
            out=xout16[3:4, :],
            in_=xxh2_sb[:].rearrange("p nl b d -> p (nl b d)"))
        nc.sync.dma_start(
            out=xout16[4:7, :],
            in_=xl3_sb[:].rearrange("p nl b d -> p (nl b d)"))
        nc.sync.dma_start(
            out=xout16[7:8, :],
            in_=xxl2_sb[:].rearrange("p nl b d -> p (nl b d)"))
    return nc


# --------------------------------------------------------------------------
# Stage B
# --------------------------------------------------------------------------
DEBUG_B = False
NF3 = 256   # level-3 fold slots


def build_stage_b():
    nc = bacc.Bacc(None)
    # k rows: 0-2 xh_j, 3-5 xl_j, 6-8 xh_j, 9 xxh2_j, 10 xxl2_j, 11 = -1
    vall = nc.declare_dram_parameter("vall", [12, B * N], FP16,
                                     isOutput=False)
    # k rows: 0-2 xh_n, 3-5 xh_n, 6-8 xl_n, 9-10 = -1, 11 xxh2_n
    uvl = nc.declare_dram_parameter("uvl", [12, NP_CORE], FP16,
                                    isOutput=False)
    rowdat = nc.declare_dram_parameter("rowdat", [128, 32 * 8], FP,
                                       isOutput=False)
    # ptab8[b][s, (u*4+q)*8 + c] = point j = s + 256u + 512q, vals
    # [x0, x1, x2, xx2, y0, y1, y2, 0]
    ptabs = [nc.declare_dram_parameter(f"ptab{b}", [NF3, 64], FP,
                                       isOutput=False) for b in range(B)]
    outc = nc.declare_dram_parameter("outc", [B, RLOC, OUT_F], FP,
                                     isOutput=True)
    if DEBUG_B:
        dbg_m3 = nc.declare_dram_parameter("dbg_m3", [128, NF3], FP16,
                                           isOutput=True)
        dbg_idx = nc.declare_dram_parameter("dbg_idx", [128, K], mybir.dt.uint16,
                                            isOutput=True)
        dbg_gth = nc.declare_dram_parameter("dbg_gth", [128, 8 * 64], FP,
                                            isOutput=True)
        dbg_pdc = nc.declare_dram_parameter("dbg_pdc", [128, K * 8], FP,
                                            isOutput=True)
        dbg_fr = nc.declare_dram_parameter("dbg_fr", [128, 32 * OUT_F], FP,
                                           isOutput=True)

    from concourse import library_config
    U16 = mybir.dt.uint16
    I16 = mybir.dt.int16

    with tile.TileContext(nc) as tc, ExitStack() as ctx:
        sbp = ctx.enter_context(tc.tile_pool(name="sbuf", bufs=1))
        lop = ctx.enter_context(tc.tile_pool(name="loop", bufs=3))
        bp = ctx.enter_context(tc.tile_pool(name="bloop", bufs=2))
        dramp = ctx.enter_context(tc.tile_pool(name="dram", bufs=2,
                                               space="DRAM"))
        pspd = ctx.enter_context(tc.tile_pool(name="pspd", bufs=2,
                                              space="PSUM"))

        nc.gpsimd.load_library(library_config.mlp)
        vall_sb = sbp.tile([12, B, N], FP16)
        nc.sync.dma_start(out=vall_sb[:],
                          in_=vall[:].rearrange("p (b n) -> p b n", b=B))
        uvl_sb = sbp.tile([12, NP_CORE], FP16)
        nc.sync.dma_start(out=uvl_sb[:], in_=uvl[:])
        rd_sb = sbp.tile([128, 32, 8], FP)
        nc.sync.dma_start(out=rd_sb[:],
                          in_=rowdat[:].rearrange("p (t v) -> p t v", t=32))
        final_raw = sbp.tile([128, 32, OUT_F], FP)

        for b in range(B):
            idx2 = bp.tile([128, 2, K], U16, tag="idx2")
            for m in range(2):
                t = b * 2 + m
                # ---- pd matmul (K=12 fp16 split-compensated)
                ppd = pspd.tile([128, N], FP, tag="ppd")
                for ch in range(4):
                    nc.tensor.matmul(
                        out=ppd[:, ch * NF:(ch + 1) * NF],
                        lhsT=uvl_sb[:, t * 128:(t + 1) * 128],
                        rhs=vall_sb[:, b, ch * NF:(ch + 1) * NF],
                        start=True, stop=True)
                # ---- folds 2048 -> 256:
                # m3[s] = max_{u,q} pd[s + 256u + 512q]
                call = lop.tile([128, 3, NF], FP16, tag="call")
                nc.scalar.activation(
                    out=call[:].rearrange("p c s -> p (c s)"),
                    in_=ppd[:, NF:2048], func=AF.Copy)
                m1 = lop.tile([128, 2, NF], FP16, tag="m1")
                nc.vector.scalar_tensor_tensor(
                    out=m1[:, 0, :], in0=ppd[:, 0:NF], scalar=0.0,
                    in1=call[:, 1, :], op0=ALU.bypass, op1=ALU.max)
                nc.vector.scalar_tensor_tensor(
                    out=m1[:, 1, :], in0=call[:, 0, :], scalar=0.0,
                    in1=call[:, 2, :], op0=ALU.bypass, op1=ALU.max)
                m2 = lop.tile([128, NF], FP16, tag="m2")
                nc.vector.scalar_tensor_tensor(
                    out=m2[:], in0=m1[:, 0, :], scalar=0.0, in1=m1[:, 1, :],
                    op0=ALU.bypass, op1=ALU.max)
                m3 = lop.tile([128, NF3], FP16, tag="m3")
                nc.vector.scalar_tensor_tensor(
                    out=m3[:], in0=m2[:, 0:NF3], scalar=0.0,
                    in1=m2[:, NF3:NF], op0=ALU.bypass, op1=ALU.max)
                # ---- top-8 slots
                top8 = lop.tile([128, K], FP16, tag="top8")
                nc.vector.max(out=top8[:], in_=m3[:])
                nc.vector.max_index(out=idx2[:, m, :], in_max=top8[:],
                                    in_values=m3[:])
                if DEBUG_B and t == 0:
                    nc.sync.dma_start(out=dbg_m3[:], in_=m3[:])

            # ---- per-tile 1024-idx gathers (list elem i = k*128 + p ->
            # wrapped [i%16, i//16]: idxs16[r, k*8+a] = idx2[16a+r, m, k])
            gth = bp.tile([128, 16, 64], FP, tag="gth")
            for m in range(2):
                scr = dramp.tile([128, K], U16, tag=f"scr{m}")
                nc.sync.dma_start(out=scr[:], in_=idx2[:, m, :])
                idxs16 = bp.tile([128, 64], I16, tag=f"idxs16{m}")
                nc.sync.dma_start(
                    out=idxs16[0:16, :].rearrange("r (k a) -> r k a", k=K),
                    in_=scr[:].rearrange("(a r) k -> r k a", r=16)
                        .bitcast(I16))
                scr2 = dramp.tile([16, 64], U16, tag=f"scr2{m}")
                nc.sync.dma_start(out=scr2[:],
                                  in_=idxs16[0:16, :].bitcast(U16))
                nc.sync.dma_start(
                    out=idxs16[:],
                    in_=scr2[:].rearrange("r c -> (r c)").unsqueeze(0)
                        .to_broadcast([8, 16 * 64]).bitcast(I16))
                nc.gpsimd.dma_gather(
                    gth[:, m * 8:(m + 1) * 8, :], ptabs[b][:], idxs16[:],
                    1024, 1024, 64)
            if DEBUG_B and b == 0:
                nc.sync.dma_start(
                    out=dbg_idx[:], in_=idx2[:, 0, :])
                nc.sync.dma_start(
                    out=dbg_gth[:],
                    in_=gth[:, 0:8, :].rearrange("p g v -> p (g v)"))

            for m in range(2):
                t = b * 2 + m
                gt = gth[:, m * 8:(m + 1) * 8, :].rearrange(
                    "p k (e c) -> p k e c", c=8)
                # ---- exact fp32 pd for the 64 candidates (Pool chains)
                p0 = lop.tile([128, K, 8], FP, tag="p0")
                nc.gpsimd.tensor_scalar(out=p0[:], in0=gt[:, :, :, 0],
                                        scalar1=rd_sb[:, t, 0:1],
                                        scalar2=None, op0=ALU.mult)
                p1 = lop.tile([128, K, 8], FP, tag="p1")
                nc.gpsimd.tensor_scalar(out=p1[:], in0=gt[:, :, :, 1],
                                        scalar1=rd_sb[:, t, 1:2],
                                        scalar2=None, op0=ALU.mult)
                nc.gpsimd.tensor_add(out=p0[:], in0=p0[:], in1=p1[:])
                nc.gpsimd.tensor_scalar(out=p1[:], in0=gt[:, :, :, 2],
                                        scalar1=rd_sb[:, t, 2:3],
                                        scalar2=None, op0=ALU.mult)
                nc.gpsimd.tensor_add(out=p0[:], in0=p0[:], in1=p1[:])
                pdc = lop.tile([128, K, 8], FP, tag="pdc")
                nc.gpsimd.tensor_sub(out=pdc[:], in0=p0[:],
                                     in1=gt[:, :, :, 3])
                # ---- threshold at 8th largest of the 64
                t8 = lop.tile([128, K], FP, tag="t8")
                nc.vector.max(out=t8[:],
                              in_=pdc[:].rearrange("p k e -> p (k e)"))
                r = lop.tile([128, K, 8], FP, tag="r")
                nc.scalar.activation(out=r[:], in_=pdc[:], func=AF.Relu,
                                     bias=t8[:, 7:8], scale=-1.0)
                # ---- masked max of y over the selected 8
                rb = lop.tile([128, K * 8], FP, tag="rb")
                nc.gpsimd.tensor_scalar(
                    out=rb[:], in0=r[:].rearrange("p k e -> p (k e)"),
                    scalar1=-BIG, scalar2=None, op0=ALU.mult)
                ys = lop.tile([128, OUT_F, K * 8], FP, tag="ys")
                nc.gpsimd.tensor_add(
                    out=ys[:],
                    in0=rb[:].unsqueeze(1).to_broadcast([128, OUT_F, K * 8]),
                    in1=gt[:, :, :, 4:7].rearrange("p k e v -> p v (k e)"))
                nc.vector.tensor_reduce(
                    out=final_raw[:, t, :], in_=ys[:],
                    axis=mybir.AxisListType.X, op=ALU.max)
                if DEBUG_B and t == 0:
                    nc.sync.dma_start(
                        out=dbg_pdc[:],
                        in_=pdc[:].rearrange("p k e -> p (k e)"))

        if DEBUG_B:
            nc.sync.dma_start(
                out=dbg_fr[:],
                in_=final_raw[:].rearrange("p t o -> p (t o)"))

        # ---- epilogue: += u, leaky, store
        nc.vector.scalar_tensor_tensor(
            out=final_raw[:], in0=final_raw[:], scalar=0.0,
            in1=rd_sb[:, :, 3:6], op0=ALU.bypass, op1=ALU.add)
        nc.vector.scalar_tensor_tensor(
            out=final_raw[:], in0=final_raw[:], scalar=0.2,
            in1=final_raw[:], op0=ALU.mult, op1=ALU.max)
        nc.sync.dma_start(
            out=outc[:].rearrange("b (m p) o -> p (b m) o", p=128),
            in_=final_raw[:])
    return nc


# --------------------------------------------------------------------------
# Host orchestration
# --------------------------------------------------------------------------
_CACHE = {}
LAST_RESULTS = {}


def _programs():
    if "a" not in _CACHE:
        nca = build_stage_a()
        nca.compile()
        ncb = build_stage_b()
        ncb.compile()
        _CACHE["a"] = nca
        _CACHE["b"] = ncb
    return _CACHE["a"], _CACHE["b"]


def _weight_folds(inputs):
    c1w = np.asarray(inputs["c1w"], np.float32)
    c1b = np.asarray(inputs["c1b"], np.float32)
    c2w = np.asarray(inputs["c2w"], np.float32)
    c2b = np.asarray(inputs["c2b"], np.float32)
    M1 = c1w[:, :3].T @ c2w.T                      # (3, 3)
    M2 = c1w[:, 3:].T @ c2w.T                      # (3, 3)
    zc = (c1b @ c2w.T + c2b).reshape(3)
    Wl = (np.asarray(inputs["Wl1"], np.float32)
          @ np.asarray(inputs["Wl2"], np.float32))  # (128, 3)
    wbig = np.zeros((IN_F, 16), np.float32)
    wbig[:, 0:3] = Wl
    wbig[:, 4:7] = Wl @ M1
    wbig[:, 8:11] = Wl @ (M2 - M1)
    abig = np.zeros((OUT_F, 16), np.float32)
    abig[:, 0:3] = np.eye(3, dtype=np.float32)
    abig[:, 4:7] = M1
    abig[:, 8:11] = M2 - M1
    bsel = np.zeros((OUT_F, 16), np.float32)
    bsel[:, 8:11] = np.eye(3, dtype=np.float32)
    biasd = np.asarray(inputs["bias"], np.float32).reshape(DEG, OUT_F)
    biasu = np.ascontiguousarray((biasd + zc.reshape(1, 3)).T)  # (3, 64)
    return wbig, abig, bsel, biasu


def _stage_a_inmaps(inputs):
    trees = [np.asarray(inputs[f"t{i}"], np.float32) for i in range(6)]
    wrs = [np.asarray(inputs[f"Wr{i}"], np.float32) for i in range(6)]
    wbf = np.asarray(inputs["W_branch"], np.float32).astype(np.float16)
    wbig, abig, bsel, biasu = _weight_folds(inputs)
    t5 = trees[5]
    in_maps = []
    for c in range(NCORES):
        m = {}
        nodes = [NLOC * c + j for j in range(NLOC)]
        for i in range(6):
            f = FEATS[i]
            nch = NCH[i]
            rows = [n * SIZES[i] // NODE for n in nodes]
            sl = trees[i][:, rows, :].transpose(2, 1, 0).reshape(f, NLOC * B)
            slp = np.zeros((nch * 128, NLOC * B), np.float32)
            slp[:f] = sl
            m[f"tlT{i}"] = np.ascontiguousarray(
                slp.reshape(nch, 128, NLOC * B).transpose(1, 0, 2)
                .reshape(128, nch * NLOC * B))
            wp = np.zeros((nch * 128, OUT_F), np.float32)
            wp[:f] = wrs[i]
            m[f"wr{i}"] = np.ascontiguousarray(
                wp.reshape(nch, 128, OUT_F).transpose(1, 0, 2)
                .reshape(128, nch * OUT_F))
        m["wb"] = np.ascontiguousarray(wbf[nodes])
        m["t5h"] = np.ascontiguousarray(
            t5[:, nodes, :].transpose(2, 1, 0)).astype(np.float16)
        m["wbig"] = wbig.astype(np.float16)
        m["abig"] = abig.astype(np.float16)
        m["bsel"] = bsel.astype(np.float16)
        m["biasu"] = biasu.astype(np.float16)
        in_maps.append(m)
    return in_maps


def _stage_b_inmaps(inputs, xouts, xout16s):
    # xouts: per-core (16, 4096) fp32; xout16s: per-core (8, 4096) fp16
    # cols (nl, b, d); global j = c*256 + nl*64 + d
    xs = np.stack([np.asarray(x).reshape(16, NLOC, B, DEG) for x in xouts])
    hs = np.stack([np.asarray(x).reshape(8, NLOC, B, DEG) for x in xout16s])
    allp = xs.transpose(1, 3, 0, 2, 4).reshape(16, B, N)     # fp32
    allh = hs.transpose(1, 3, 0, 2, 4).reshape(8, B, N)      # fp16
    # vall rows: xh, xl, xh, xxh2, xxl2, -1
    vall = np.empty((12, B, N), np.float16)
    vall[0:3] = allh[0:3]
    vall[3:6] = allh[4:7]
    vall[6:9] = allh[0:3]
    vall[9] = allh[3]
    vall[10] = allh[7]
    vall[11] = -1.0
    vall = np.ascontiguousarray(vall.reshape(12, B * N))
    ptabs = {}
    for b in range(B):
        pt = np.zeros((N, 8), np.float32)
        pt[:, 0:7] = allp[0:7, b].T       # x(3), xx2, y(3)
        # ptab8[s, e=(u*4+q), :] = pt[s + 256u + 512q]
        p8 = pt.reshape(2, 4, 256, 8).transpose(2, 0, 1, 3)  # (s, u, q, 8)
        ptabs[f"ptab{b}"] = np.ascontiguousarray(p8.reshape(256, 64))
    in_maps = []
    for c in range(NCORES):
        own = xs[c].transpose(0, 2, 1, 3).reshape(16, B, RLOC)
        ownh = hs[c].transpose(0, 2, 1, 3).reshape(8, NP_CORE)
        u = np.empty((12, NP_CORE), np.float16)
        u[0:3] = ownh[0:3]
        u[3:6] = ownh[0:3]
        u[6:9] = ownh[4:7]
        u[9:11] = -1.0
        u[11] = ownh[3]
        rd = np.zeros((128, 32, 8), np.float32)
        o2 = own.reshape(16, B, 2, 128)
        rd[:, :, 0:3] = o2[0:3].transpose(3, 1, 2, 0).reshape(128, 32, 3)
        rd[:, :, 3:6] = o2[8:11].transpose(3, 1, 2, 0).reshape(128, 32, 3)
        m = {"vall": vall, "uvl": np.ascontiguousarray(u),
             "rowdat": np.ascontiguousarray(rd.reshape(128, 32 * 8))}
        m.update(ptabs)
        in_maps.append(m)
    return in_maps


def kernel(**inputs):
    nca, ncb = _programs()
    core_ids = list(range(NCORES))

    ra = run_bass_kernel_spmd(nca, _stage_a_inmaps(inputs), core_ids)
    LAST_RESULTS["a"] = ra
    xouts = [np.asarray(ra.results[c]["xout"]) for c in range(NCORES)]
    xout16s = [np.asarray(ra.results[c]["xout16"]) for c in range(NCORES)]

    rb = run_bass_kernel_spmd(ncb, _stage_b_inmaps(inputs, xouts, xout16s),
                              core_ids)
    LAST_RESULTS["b"] = rb
    out = np.empty((B, N, OUT_F), np.float32)
    for c in range(NCORES):
        out[:, c * RLOC:(c + 1) * RLOC, :] = rb.results[c]["outc"]
    return out


# revision 26
# speedup vs baseline: 1.8301x; 1.1027x over previous
"""Trainium2 Bass kernel for nn_BranchGCN (gnn_message_passing).

Two SPMD launches over 8 cores:
  Stage A -- node-model-parallel: core c owns nodes [4c, 4c+4) and streams its
    1/8 slice of W_branch (fp16) while computing per point: x, xx2=|x|^2/2,
    y = M1^T x, u = (M2-M1)^T x + bias + zc, plus an error-compensated fp16
    split (xh + xl) of x and xx2 for stage B's distance matmul. Weight-only
    folds (Wl1@Wl2, conv factorization M1/M2/zc) are host precomputed; all
    per-sample math stays on device.
  Host    -- pure relayout (numpy transposes / dtype casts / constant fills).
  Stage B -- row-sharded KNN EdgeConv. Per 128-row tile, one K=12 fp16 matmul
    (split-compensated, fp32-grade accuracy) yields
      pd[n,j] = x_n.x_j - |x_j|^2/2 - |x_n|^2/2  (~ -|x_n-x_j|^2/2, row-shift)
    Exact top-8 via two pairwise-max folds (2048->512 slots), max8+max_index
    on the folded array, then 4-way slot expansion: the true top-8 provably
    lies among the 32 expanded candidates. One multi-offset indirect DMA
    gathers their [x, xx2, y] rows, pd is recomputed in fp32 for the 32,
    thresholded at the 8th largest, and max_k y + center term u -> leaky.
"""

import sys
import numpy as np

sys.path.insert(0, "/opt/trn_rl_repo")

from contextlib import ExitStack

import concourse.tile as tile
from concourse import bacc, bass, mybir
from concourse.bass_utils import run_bass_kernel_spmd

FP = mybir.dt.float32
FP16 = mybir.dt.float16
U32 = mybir.dt.uint32
ALU = mybir.AluOpType
AF = mybir.ActivationFunctionType

B, NODE, DEG, K = 16, 32, 64, 8
IN_F, OUT_F, SUP = 128, 3, 10
FEATS = [96, 256, 256, 256, 128, 128]
SIZES = [1, 2, 4, 8, 16, 32]
NCORES = 8
NLOC = NODE // NCORES          # 4 nodes per core
N = NODE * DEG                 # 2048 graph rows
RLOC = NLOC * DEG              # 256 rows per core
NP_CORE = B * RLOC             # 4096 points per core
NF = 512                       # fold slots (2048 / 4)
BIG = 60000.0
NCH = [(f + 127) // 128 for f in FEATS]


# --------------------------------------------------------------------------
# Stage A
# --------------------------------------------------------------------------
def build_stage_a():
    nc = bacc.Bacc(None)
    tlT = [nc.declare_dram_parameter(f"tlT{i}", [128, NCH[i] * NLOC * B], FP,
                                     isOutput=False) for i in range(6)]
    wrs = [nc.declare_dram_parameter(f"wr{i}", [128, NCH[i] * OUT_F], FP,
                                     isOutput=False) for i in range(6)]
    wb = nc.declare_dram_parameter("wb", [NLOC, IN_F, DEG * IN_F], FP16,
                                   isOutput=False)
    t5h = nc.declare_dram_parameter("t5h", [IN_F, NLOC, B], FP16,
                                    isOutput=False)
    wbig = nc.declare_dram_parameter("wbig", [IN_F, 16], FP16, isOutput=False)
    abig = nc.declare_dram_parameter("abig", [OUT_F, 16], FP16, isOutput=False)
    bsel = nc.declare_dram_parameter("bsel", [OUT_F, 16], FP16, isOutput=False)
    biasu = nc.declare_dram_parameter("biasu", [OUT_F, DEG], FP16,
                                      isOutput=False)
    # fp32 rows: 0-2 x, 3 xx2, 4-6 y, 8-10 u; cols (nl, b, d)
    xout = nc.declare_dram_parameter("xout", [16, NP_CORE], FP, isOutput=True)
    # fp16 rows: 0-2 xh, 3 xxh2, 4-6 xl, 7 xxl2
    xout16 = nc.declare_dram_parameter("xout16", [8, NP_CORE], FP16,
                                       isOutput=True)

    with tile.TileContext(nc) as tc, ExitStack() as ctx:
        sbp = ctx.enter_context(tc.tile_pool(name="sbuf", bufs=1))
        wbpool = ctx.enter_context(tc.tile_pool(name="wbuf", bufs=2))
        psp = ctx.enter_context(tc.tile_pool(name="psum", bufs=2,
                                             space="PSUM"))
        pbp = ctx.enter_context(tc.tile_pool(name="psumb", bufs=2,
                                             space="PSUM"))
        pxp = ctx.enter_context(tc.tile_pool(name="psumx", bufs=2,
                                             space="PSUM"))

        # ---- small loads
        tl_sb, wr_sb = [], []
        for i in range(6):
            t = sbp.tile([128, NCH[i], NLOC * B], FP, tag=f"tlT{i}")
            nc.sync.dma_start(out=t[:], in_=tlT[i][:].rearrange(
                "p (c n) -> p c n", c=NCH[i]))
            tl_sb.append(t)
            w = sbp.tile([128, NCH[i], OUT_F], FP, tag=f"wr{i}")
            nc.sync.dma_start(out=w[:], in_=wrs[i][:].rearrange(
                "p (c o) -> p c o", c=NCH[i]))
            wr_sb.append(w)
        t5_sb = sbp.tile([IN_F, NLOC, B], FP16)
        nc.sync.dma_start(out=t5_sb[:], in_=t5h[:])
        wbig_sb = sbp.tile([IN_F, 16], FP16)
        nc.sync.dma_start(out=wbig_sb[:], in_=wbig[:])
        abig_sb = sbp.tile([OUT_F, 16], FP16)
        nc.sync.dma_start(out=abig_sb[:], in_=abig[:])
        bsel_sb = sbp.tile([OUT_F, 16], FP16)
        nc.sync.dma_start(out=bsel_sb[:], in_=bsel[:])
        biasu_sb = sbp.tile([OUT_F, DEG], FP16)
        nc.sync.dma_start(out=biasu_sb[:], in_=biasu[:])
        oneh = sbp.tile([OUT_F, 1], FP)
        nc.vector.memset(oneh[:], 0.5)

        # ---- root aggregation: rootT fp16 (3, nl, b)
        proot = psp.tile([OUT_F, NLOC * B], FP, tag="proot")
        steps = [(i, c) for i in range(6) for c in range(NCH[i])]
        for si, (i, c) in enumerate(steps):
            nc.tensor.matmul(out=proot[:],
                             lhsT=wr_sb[i][:, c, :],
                             rhs=tl_sb[i][:, c, :],
                             start=(si == 0), stop=(si == len(steps) - 1))
        rootT = sbp.tile([OUT_F, NLOC, B], FP16)
        nc.scalar.activation(out=rootT[:],
                             in_=proot[:].rearrange("p (nl b) -> p nl b",
                                                    nl=NLOC),
                             func=AF.Copy)

        # ---- per-node: branch einsum (fp16) + leaky -> branchT fp16
        branchT = sbp.tile([IN_F, NLOC, B, 2, 32], FP16)
        xout_sb = sbp.tile([16, NLOC, B, DEG], FP)
        xx2_sb = sbp.tile([1, NLOC, B, DEG], FP)
        xh3_sb = sbp.tile([3, NLOC, B, DEG], FP16)
        xxh2_sb = sbp.tile([1, NLOC, B, DEG], FP16)
        xl3_sb = sbp.tile([3, NLOC, B, DEG], FP16)
        xxl2_sb = sbp.tile([1, NLOC, B, DEG], FP16)
        for nl in range(NLOC):
            wbt = wbpool.tile([IN_F, DEG * IN_F], FP16, tag="wbt")
            nc.sync.dma_start(out=wbt[:], in_=wb[nl])
            for g in range(2):
                pb = pbp.tile([IN_F, 32, B], FP, tag="pbranch")
                for dl in range(32):
                    d = g * 32 + dl
                    nc.tensor.matmul(out=pb[:, dl, :],
                                     lhsT=wbt[:, d * 128:(d + 1) * 128],
                                     rhs=t5_sb[:, nl, :],
                                     start=True, stop=True)
                # transpose-copy PSUM -> SBUF fp16 (b-major), then leaky
                cg = sbp.tile([IN_F, B, 32], FP16, tag="cg")
                nc.vector.tensor_copy(
                    out=cg[:].rearrange("p b dl -> p dl b"), in_=pb[:])
                nc.vector.scalar_tensor_tensor(
                    out=branchT[:, nl, :, g, :], in0=cg[:], scalar=0.2,
                    in1=cg[:], op0=ALU.mult, op1=ALU.max)
            # rows [x, y, u] for this node's points, 2 chunks of (8b x 64d)
            for h in range(2):
                pxo = pxp.tile([16, 8, DEG], FP, tag="pxo")
                nc.tensor.matmul(
                    out=pxo[:],
                    lhsT=wbig_sb[:],
                    rhs=branchT[:, nl, 8 * h:8 * h + 8, :, :].rearrange(
                        "p b g dl -> p (b g dl)"),
                    start=True, stop=False)
                nc.tensor.matmul(
                    out=pxo[:],
                    lhsT=abig_sb[:],
                    rhs=rootT[:, nl, 8 * h:8 * h + 8].unsqueeze(2)
                        .to_broadcast([OUT_F, 8, DEG]),
                    start=False, stop=False)
                nc.tensor.matmul(
                    out=pxo[:],
                    lhsT=bsel_sb[:],
                    rhs=biasu_sb[:].unsqueeze(1)
                        .to_broadcast([OUT_F, 8, DEG]),
                    start=False, stop=True)
                xo = xout_sb[:, nl, 8 * h:8 * h + 8, :]
                nc.scalar.activation(out=xo, in_=pxo[:], func=AF.Copy)
                # xx2 = 0.5 * sum x_c^2
                sq = sbp.tile([OUT_F, 8 * DEG], FP, tag="sq")
                nc.vector.scalar_tensor_tensor(
                    out=sq[:],
                    in0=xo[0:3].rearrange("p b d -> p (b d)"), scalar=0.0,
                    in1=xo[0:3].rearrange("p b d -> p (b d)"),
                    op0=ALU.bypass, op1=ALU.mult)
                pxx = psp.tile([1, 8 * DEG], FP, tag="pxx")
                nc.tensor.matmul(out=pxx[:], lhsT=oneh[:], rhs=sq[:],
                                 start=True, stop=True)
                nc.vector.tensor_copy(
                    out=xx2_sb[:, nl, 8 * h:8 * h + 8, :].rearrange(
                        "p b d -> p (b d)"), in_=pxx[:])
            # fp16 split of x and xx2 for this node (partition-0 aligned ops)
            nc.scalar.activation(out=xh3_sb[:, nl], in_=xout_sb[0:3, nl],
                                 func=AF.Copy)
            nc.scalar.activation(out=xxh2_sb[:, nl], in_=xx2_sb[:, nl],
                                 func=AF.Copy)
            nc.vector.scalar_tensor_tensor(
                out=xl3_sb[:, nl], in0=xout_sb[0:3, nl], scalar=0.0,
                in1=xh3_sb[:, nl], op0=ALU.bypass, op1=ALU.subtract)
            nc.vector.scalar_tensor_tensor(
                out=xxl2_sb[:, nl], in0=xx2_sb[:, nl], scalar=0.0,
                in1=xxh2_sb[:, nl], op0=ALU.bypass, op1=ALU.subtract)

        nc.sync.dma_start(
            out=xout[0:3, :],
            in_=xout_sb[0:3].rearrange("p nl b d -> p (nl b d)"))
        nc.sync.dma_start(
            out=xout[4:16, :],
            in_=xout_sb[4:16].rearrange("p nl b d -> p (nl b d)"))
        nc.sync.dma_start(
            out=xout[3:4, :],
            in_=xx2_sb[:].rearrange("p nl b d -> p (nl b d)"))
        nc.sync.dma_start(
            out=xout16[0:3, :],
            in_=xh3_sb[:].rearrange("p nl b d -> p (nl b d)"))
        nc.sync.dma_start(
            out=xout16[3:4, :],
            in_=xxh2_sb[:].rearrange("p nl b d -> p (nl b d)"))
        nc.sync.dma_start(
            out=xout16[4:7, :],
            in_=xl3_sb[:].rearrange("p nl b d -> p (nl b d)"))
        nc.sync.dma_start(
            out=xout16[7:8, :],
            in_=xxl2_sb[:].rearrange("p nl b d -> p (nl b d)"))
    return nc


# --------------------------------------------------------------------------
# Stage B
# --------------------------------------------------------------------------
DEBUG_B = False
NF3 = 256   # level-3 fold slots


def build_stage_b():
    nc = bacc.Bacc(None)
    # k rows: 0-2 xh_j, 3-5 xl_j, 6-8 xh_j, 9 xxh2_j, 10 xxl2_j, 11 = -1
    vall = nc.declare_dram_parameter("vall", [12, B * N], FP16,
                                     isOutput=False)
    # k rows: 0-2 xh_n, 3-5 xh_n, 6-8 xl_n, 9-10 = -1, 11 xxh2_n
    uvl = nc.declare_dram_parameter("uvl", [12, NP_CORE], FP16,
                                    isOutput=False)
    rowdat = nc.declare_dram_parameter("rowdat", [128, 32 * 8], FP,
                                       isOutput=False)
    # ptab8[b][s, (u*4+q)*8 + c] = point j = s + 256u + 512q, vals
    # [x0, x1, x2, xx2, y0, y1, y2, 0]
    ptabs = [nc.declare_dram_parameter(f"ptab{b}", [NF3, 64], FP,
                                       isOutput=False) for b in range(B)]
    outc = nc.declare_dram_parameter("outc", [B, RLOC, OUT_F], FP,
                                     isOutput=True)
    if DEBUG_B:
        dbg_m3 = nc.declare_dram_parameter("dbg_m3", [128, NF3], FP16,
                                           isOutput=True)
        dbg_idx = nc.declare_dram_parameter("dbg_idx", [128, K], mybir.dt.uint16,
                                            isOutput=True)
        dbg_gth = nc.declare_dram_parameter("dbg_gth", [128, 8 * 64], FP,
                                            isOutput=True)
        dbg_pdc = nc.declare_dram_parameter("dbg_pdc", [128, K * 8], FP,
                                            isOutput=True)
        dbg_fr = nc.declare_dram_parameter("dbg_fr", [128, 32 * OUT_F], FP,
                                           isOutput=True)

    from concourse import library_config
    U16 = mybir.dt.uint16
    I16 = mybir.dt.int16

    with tile.TileContext(nc) as tc, ExitStack() as ctx:
        sbp = ctx.enter_context(tc.tile_pool(name="sbuf", bufs=1))
        lop = ctx.enter_context(tc.tile_pool(name="loop", bufs=3))
        bp = ctx.enter_context(tc.tile_pool(name="bloop", bufs=2))
        dramp = ctx.enter_context(tc.tile_pool(name="dram", bufs=2,
                                               space="DRAM"))
        pspd = ctx.enter_context(tc.tile_pool(name="pspd", bufs=2,
                                              space="PSUM"))

        nc.gpsimd.load_library(library_config.mlp)
        vall_sb = sbp.tile([12, B, N], FP16)
        nc.sync.dma_start(out=vall_sb[:],
                          in_=vall[:].rearrange("p (b n) -> p b n", b=B))
        uvl_sb = sbp.tile([12, NP_CORE], FP16)
        nc.sync.dma_start(out=uvl_sb[:], in_=uvl[:])
        rd_sb = sbp.tile([128, 32, 8], FP)
        nc.sync.dma_start(out=rd_sb[:],
                          in_=rowdat[:].rearrange("p (t v) -> p t v", t=32))
        final_raw = sbp.tile([128, 32, OUT_F], FP)

        for b in range(B):
            idx2 = bp.tile([128, 2, K], U16, tag="idx2")
            for m in range(2):
                t = b * 2 + m
                # ---- pd matmul (K=12 fp16 split-compensated)
                ppd = pspd.tile([128, N], FP, tag="ppd")
                for ch in range(4):
                    nc.tensor.matmul(
                        out=ppd[:, ch * NF:(ch + 1) * NF],
                        lhsT=uvl_sb[:, t * 128:(t + 1) * 128],
                        rhs=vall_sb[:, b, ch * NF:(ch + 1) * NF],
                        start=True, stop=True)
                # ---- folds 2048 -> 256:
                # m3[s] = max_{u,q} pd[s + 256u + 512q]
                call = lop.tile([128, 3, NF], FP16, tag="call")
                nc.scalar.activation(
                    out=call[:].rearrange("p c s -> p (c s)"),
                    in_=ppd[:, NF:2048], func=AF.Copy)
                m1 = lop.tile([128, 2, NF], FP16, tag="m1")
                nc.vector.tensor_tensor(
                    out=m1[:, 0, :], in0=ppd[:, 0:NF],
                    in1=call[:, 1, :], op=ALU.max)
                nc.vector.tensor_tensor(
                    out=m1[:, 1, :], in0=call[:, 0, :],
                    in1=call[:, 2, :], op=ALU.max)
                m2 = lop.tile([128, NF], FP16, tag="m2")
                nc.vector.tensor_tensor(
                    out=m2[:], in0=m1[:, 0, :], in1=m1[:, 1, :], op=ALU.max)
                m3 = lop.tile([128, NF3], FP16, tag="m3")
                nc.vector.tensor_tensor(
                    out=m3[:], in0=m2[:, 0:NF3], in1=m2[:, NF3:NF],
                    op=ALU.max)
                # ---- top-8 slots
                top8 = lop.tile([128, K], FP16, tag="top8")
                nc.vector.max(out=top8[:], in_=m3[:])
                nc.vector.max_index(out=idx2[:, m, :], in_max=top8[:],
                                    in_values=m3[:])
                if DEBUG_B and t == 0:
                    nc.sync.dma_start(out=dbg_m3[:], in_=m3[:])

            # ---- per-tile 1024-idx gathers (list elem i = k*128 + p ->
            # wrapped [i%16, i//16]: idxs16[r, k*8+a] = idx2[16a+r, m, k])
            gth = bp.tile([128, 16, 64], FP, tag="gth")
            idxs16 = bp.tile([128, 2, 64], I16, tag="idxs16")
            for m in range(2):
                scr = dramp.tile([128, K], U16, tag=f"scr{m}")
                nc.sync.dma_start(out=scr[:], in_=idx2[:, m, :])
                nc.sync.dma_start(
                    out=idxs16[0:16, m, :].rearrange("r (k a) -> r k a", k=K),
                    in_=scr[:].rearrange("(a r) k -> r k a", r=16)
                        .bitcast(I16))
            scr2 = dramp.tile([16, 128], U16, tag="scr2")
            nc.sync.dma_start(
                out=scr2[:],
                in_=idxs16[0:16, :, :].rearrange("r m c -> r (m c)")
                    .bitcast(U16))
            nc.sync.dma_start(
                out=idxs16[:].rearrange("p m c -> p (m c)"),
                in_=scr2[:].rearrange("r c -> (r c)").unsqueeze(0)
                    .to_broadcast([8, 16 * 128]).bitcast(I16))
            for m in range(2):
                nc.gpsimd.dma_gather(
                    gth[:, m * 8:(m + 1) * 8, :], ptabs[b][:],
                    idxs16[:, m, :], 1024, 1024, 64)
            if DEBUG_B and b == 0:
                nc.sync.dma_start(
                    out=dbg_idx[:], in_=idx2[:, 0, :])
                nc.sync.dma_start(
                    out=dbg_gth[:],
                    in_=gth[:, 0:8, :].rearrange("p g v -> p (g v)"))

            for m in range(2):
                t = b * 2 + m
                gt = gth[:, m * 8:(m + 1) * 8, :].rearrange(
                    "p k (e c) -> p k e c", c=8)
                # ---- exact fp32 pd for the 64 candidates (Pool chains)
                p0 = lop.tile([128, K, 8], FP, tag="p0")
                nc.gpsimd.tensor_scalar(out=p0[:], in0=gt[:, :, :, 0],
                                        scalar1=rd_sb[:, t, 0:1],
                                        scalar2=None, op0=ALU.mult)
                p1 = lop.tile([128, K, 8], FP, tag="p1")
                nc.gpsimd.tensor_scalar(out=p1[:], in0=gt[:, :, :, 1],
                                        scalar1=rd_sb[:, t, 1:2],
                                        scalar2=None, op0=ALU.mult)
                nc.gpsimd.tensor_add(out=p0[:], in0=p0[:], in1=p1[:])
                nc.gpsimd.tensor_scalar(out=p1[:], in0=gt[:, :, :, 2],
                                        scalar1=rd_sb[:, t, 2:3],
                                        scalar2=None, op0=ALU.mult)
                nc.gpsimd.tensor_add(out=p0[:], in0=p0[:], in1=p1[:])
                pdc = lop.tile([128, K, 8], FP, tag="pdc")
                nc.gpsimd.tensor_sub(out=pdc[:], in0=p0[:],
                                     in1=gt[:, :, :, 3])
                # ---- threshold at 8th largest of the 64
                t8 = lop.tile([128, K], FP, tag="t8")
                nc.vector.max(out=t8[:],
                              in_=pdc[:].rearrange("p k e -> p (k e)"))
                r = lop.tile([128, K, 8], FP, tag="r")
                nc.scalar.activation(out=r[:], in_=pdc[:], func=AF.Relu,
                                     bias=t8[:, 7:8], scale=-1.0)
                # ---- masked max of y over the selected 8
                rb = lop.tile([128, K * 8], FP, tag="rb")
                nc.gpsimd.tensor_scalar(
                    out=rb[:], in0=r[:].rearrange("p k e -> p (k e)"),
                    scalar1=-BIG, scalar2=None, op0=ALU.mult)
                ys = lop.tile([128, OUT_F, K * 8], FP, tag="ys")
                nc.gpsimd.tensor_add(
                    out=ys[:],
                    in0=rb[:].unsqueeze(1).to_broadcast([128, OUT_F, K * 8]),
                    in1=gt[:, :, :, 4:7].rearrange("p k e v -> p v (k e)"))
                nc.vector.tensor_reduce(
                    out=final_raw[:, t, :], in_=ys[:],
                    axis=mybir.AxisListType.X, op=ALU.max)
                if DEBUG_B and t == 0:
                    nc.sync.dma_start(
                        out=dbg_pdc[:],
                        in_=pdc[:].rearrange("p k e -> p (k e)"))

        if DEBUG_B:
            nc.sync.dma_start(
                out=dbg_fr[:],
                in_=final_raw[:].rearrange("p t o -> p (t o)"))

        # ---- epilogue: += u, leaky, store
        nc.vector.scalar_tensor_tensor(
            out=final_raw[:], in0=final_raw[:], scalar=0.0,
            in1=rd_sb[:, :, 3:6], op0=ALU.bypass, op1=ALU.add)
        nc.vector.scalar_tensor_tensor(
            out=final_raw[:], in0=final_raw[:], scalar=0.2,
            in1=final_raw[:], op0=ALU.mult, op1=ALU.max)
        nc.sync.dma_start(
            out=outc[:].rearrange("b (m p) o -> p (b m) o", p=128),
            in_=final_raw[:])
    return nc


# --------------------------------------------------------------------------
# Host orchestration
# --------------------------------------------------------------------------
_CACHE = {}
LAST_RESULTS = {}


def _programs():
    if "a" not in _CACHE:
        nca = build_stage_a()
        nca.compile()
        ncb = build_stage_b()
        ncb.compile()
        _CACHE["a"] = nca
        _CACHE["b"] = ncb
    return _CACHE["a"], _CACHE["b"]


def _weight_folds(inputs):
    c1w = np.asarray(inputs["c1w"], np.float32)
    c1b = np.asarray(inputs["c1b"], np.float32)
    c2w = np.asarray(inputs["c2w"], np.float32)
    c2b = np.asarray(inputs["c2b"], np.float32)
    M1 = c1w[:, :3].T @ c2w.T                      # (3, 3)
    M2 = c1w[:, 3:].T @ c2w.T                      # (3, 3)
    zc = (c1b @ c2w.T + c2b).reshape(3)
    Wl = (np.asarray(inputs["Wl1"], np.float32)
          @ np.asarray(inputs["Wl2"], np.float32))  # (128, 3)
    wbig = np.zeros((IN_F, 16), np.float32)
    wbig[:, 0:3] = Wl
    wbig[:, 4:7] = Wl @ M1
    wbig[:, 8:11] = Wl @ (M2 - M1)
    abig = np.zeros((OUT_F, 16), np.float32)
    abig[:, 0:3] = np.eye(3, dtype=np.float32)
    abig[:, 4:7] = M1
    abig[:, 8:11] = M2 - M1
    bsel = np.zeros((OUT_F, 16), np.float32)
    bsel[:, 8:11] = np.eye(3, dtype=np.float32)
    biasd = np.asarray(inputs["bias"], np.float32).reshape(DEG, OUT_F)
    biasu = np.ascontiguousarray((biasd + zc.reshape(1, 3)).T)  # (3, 64)
    return wbig, abig, bsel, biasu


def _stage_a_inmaps(inputs):
    trees = [np.asarray(inputs[f"t{i}"], np.float32) for i in range(6)]
    wrs = [np.asarray(inputs[f"Wr{i}"], np.float32) for i in range(6)]
    wbf = np.asarray(inputs["W_branch"], np.float32).astype(np.float16)
    wbig, abig, bsel, biasu = _weight_folds(inputs)
    t5 = trees[5]
    in_maps = []
    for c in range(NCORES):
        m = {}
        nodes = [NLOC * c + j for j in range(NLOC)]
        for i in range(6):
            f = FEATS[i]
            nch = NCH[i]
            rows = [n * SIZES[i] // NODE for n in nodes]
            sl = trees[i][:, rows, :].transpose(2, 1, 0).reshape(f, NLOC * B)
            slp = np.zeros((nch * 128, NLOC * B), np.float32)
            slp[:f] = sl
            m[f"tlT{i}"] = np.ascontiguousarray(
                slp.reshape(nch, 128, NLOC * B).transpose(1, 0, 2)
                .reshape(128, nch * NLOC * B))
            wp = np.zeros((nch * 128, OUT_F), np.float32)
            wp[:f] = wrs[i]
            m[f"wr{i}"] = np.ascontiguousarray(
                wp.reshape(nch, 128, OUT_F).transpose(1, 0, 2)
                .reshape(128, nch * OUT_F))
        m["wb"] = np.ascontiguousarray(wbf[nodes])
        m["t5h"] = np.ascontiguousarray(
            t5[:, nodes, :].transpose(2, 1, 0)).astype(np.float16)
        m["wbig"] = wbig.astype(np.float16)
        m["abig"] = abig.astype(np.float16)
        m["bsel"] = bsel.astype(np.float16)
        m["biasu"] = biasu.astype(np.float16)
        in_maps.append(m)
    return in_maps


def _stage_b_inmaps(inputs, xouts, xout16s):
    # xouts: per-core (16, 4096) fp32; xout16s: per-core (8, 4096) fp16
    # cols (nl, b, d); global j = c*256 + nl*64 + d
    xs = np.stack([np.asarray(x).reshape(16, NLOC, B, DEG) for x in xouts])
    hs = np.stack([np.asarray(x).reshape(8, NLOC, B, DEG) for x in xout16s])
    allp = xs.transpose(1, 3, 0, 2, 4).reshape(16, B, N)     # fp32
    allh = hs.transpose(1, 3, 0, 2, 4).reshape(8, B, N)      # fp16
    # vall rows: xh, xl, xh, xxh2, xxl2, -1
    vall = np.empty((12, B, N), np.float16)
    vall[0:3] = allh[0:3]
    vall[3:6] = allh[4:7]
    vall[6:9] = allh[0:3]
    vall[9] = allh[3]
    vall[10] = allh[7]
    vall[11] = -1.0
    vall = np.ascontiguousarray(vall.reshape(12, B * N))
    ptabs = {}
    for b in range(B):
        pt = np.zeros((N, 8), np.float32)
        pt[:, 0:7] = allp[0:7, b].T       # x(3), xx2, y(3)
        # ptab8[s, e=(u*4+q), :] = pt[s + 256u + 512q]
        p8 = pt.reshape(2, 4, 256, 8).transpose(2, 0, 1, 3)  # (s, u, q, 8)
        ptabs[f"ptab{b}"] = np.ascontiguousarray(p8.reshape(256, 64))
    in_maps = []
    for c in range(NCORES):
        own = xs[c].transpose(0, 2, 1, 3).reshape(16, B, RLOC)
        ownh = hs[c].transpose(0, 2, 1, 3).reshape(8, NP_CORE)
        u = np.empty((12, NP_CORE), np.float16)
        u[0:3] = ownh[0:3]
        u[3:6] = ownh[0:3]
        u[6:9] = ownh[4:7]
        u[9:11] = -1.0
        u[11] = ownh[3]
        rd = np.zeros((128, 32, 8), np.float32)
        o2 = own.reshape(16, B, 2, 128)
        rd[:, :, 0:3] = o2[0:3].transpose(3, 1, 2, 0).reshape(128, 32, 3)
        rd[:, :, 3:6] = o2[8:11].transpose(3, 1, 2, 0).reshape(128, 32, 3)
        m = {"vall": vall, "uvl": np.ascontiguousarray(u),
             "rowdat": np.ascontiguousarray(rd.reshape(128, 32 * 8))}
        m.update(ptabs)
        in_maps.append(m)
    return in_maps


def kernel(**inputs):
    nca, ncb = _programs()
    core_ids = list(range(NCORES))

    ra = run_bass_kernel_spmd(nca, _stage_a_inmaps(inputs), core_ids)
    LAST_RESULTS["a"] = ra
    xouts = [np.asarray(ra.results[c]["xout"]) for c in range(NCORES)]
    xout16s = [np.asarray(ra.results[c]["xout16"]) for c in range(NCORES)]

    rb = run_bass_kernel_spmd(ncb, _stage_b_inmaps(inputs, xouts, xout16s),
                              core_ids)
    LAST_RESULTS["b"] = rb
    out = np.empty((B, N, OUT_F), np.float32)
    for c in range(NCORES):
        out[:, c * RLOC:(c + 1) * RLOC, :] = rb.results[c]["outc"]
    return out


# revision 27
# speedup vs baseline: 1.8396x; 1.0052x over previous
"""Trainium2 Bass kernel for nn_BranchGCN (gnn_message_passing).

Two SPMD launches over 8 cores:
  Stage A -- node-model-parallel: core c owns nodes [4c, 4c+4) and streams its
    1/8 slice of W_branch (fp16) while computing per point: x, xx2=|x|^2/2,
    y = M1^T x, u = (M2-M1)^T x + bias + zc, plus an error-compensated fp16
    split (xh + xl) of x and xx2 for stage B's distance matmul. Weight-only
    folds (Wl1@Wl2, conv factorization M1/M2/zc) are host precomputed; all
    per-sample math stays on device.
  Host    -- pure relayout (numpy transposes / dtype casts / constant fills).
  Stage B -- row-sharded KNN EdgeConv. Per 128-row tile, one K=12 fp16 matmul
    (split-compensated, fp32-grade accuracy) yields
      pd[n,j] = x_n.x_j - |x_j|^2/2 - |x_n|^2/2  (~ -|x_n-x_j|^2/2, row-shift)
    Exact top-8 via two pairwise-max folds (2048->512 slots), max8+max_index
    on the folded array, then 4-way slot expansion: the true top-8 provably
    lies among the 32 expanded candidates. One multi-offset indirect DMA
    gathers their [x, xx2, y] rows, pd is recomputed in fp32 for the 32,
    thresholded at the 8th largest, and max_k y + center term u -> leaky.
"""

import sys
import numpy as np

sys.path.insert(0, "/opt/trn_rl_repo")

from contextlib import ExitStack

import concourse.tile as tile
from concourse import bacc, bass, mybir
from concourse.bass_utils import run_bass_kernel_spmd

FP = mybir.dt.float32
FP16 = mybir.dt.float16
U32 = mybir.dt.uint32
ALU = mybir.AluOpType
AF = mybir.ActivationFunctionType

B, NODE, DEG, K = 16, 32, 64, 8
IN_F, OUT_F, SUP = 128, 3, 10
FEATS = [96, 256, 256, 256, 128, 128]
SIZES = [1, 2, 4, 8, 16, 32]
NCORES = 8
NLOC = NODE // NCORES          # 4 nodes per core
N = NODE * DEG                 # 2048 graph rows
RLOC = NLOC * DEG              # 256 rows per core
NP_CORE = B * RLOC             # 4096 points per core
NF = 512                       # fold slots (2048 / 4)
BIG = 60000.0
NCH = [(f + 127) // 128 for f in FEATS]


# --------------------------------------------------------------------------
# Stage A
# --------------------------------------------------------------------------
def build_stage_a():
    nc = bacc.Bacc(None)
    tlT = [nc.declare_dram_parameter(f"tlT{i}", [128, NCH[i] * NLOC * B], FP,
                                     isOutput=False) for i in range(6)]
    wrs = [nc.declare_dram_parameter(f"wr{i}", [128, NCH[i] * OUT_F], FP,
                                     isOutput=False) for i in range(6)]
    wb = nc.declare_dram_parameter("wb", [NLOC, IN_F, DEG * IN_F], FP16,
                                   isOutput=False)
    t5h = nc.declare_dram_parameter("t5h", [IN_F, NLOC, B], FP16,
                                    isOutput=False)
    wbig = nc.declare_dram_parameter("wbig", [IN_F, 16], FP16, isOutput=False)
    abig = nc.declare_dram_parameter("abig", [OUT_F, 16], FP16, isOutput=False)
    bsel = nc.declare_dram_parameter("bsel", [OUT_F, 16], FP16, isOutput=False)
    biasu = nc.declare_dram_parameter("biasu", [OUT_F, DEG], FP16,
                                      isOutput=False)
    # fp32 rows: 0-2 x, 3 xx2, 4-6 y, 8-10 u; cols (nl, b, d)
    xout = nc.declare_dram_parameter("xout", [16, NP_CORE], FP, isOutput=True)
    # fp16 rows: 0-2 xh, 3 xxh2, 4-6 xl, 7 xxl2
    xout16 = nc.declare_dram_parameter("xout16", [8, NP_CORE], FP16,
                                       isOutput=True)

    with tile.TileContext(nc) as tc, ExitStack() as ctx:
        sbp = ctx.enter_context(tc.tile_pool(name="sbuf", bufs=1))
        wbpool = ctx.enter_context(tc.tile_pool(name="wbuf", bufs=2))
        psp = ctx.enter_context(tc.tile_pool(name="psum", bufs=2,
                                             space="PSUM"))
        pbp = ctx.enter_context(tc.tile_pool(name="psumb", bufs=2,
                                             space="PSUM"))
        pxp = ctx.enter_context(tc.tile_pool(name="psumx", bufs=2,
                                             space="PSUM"))

        # ---- small loads
        tl_sb, wr_sb = [], []
        for i in range(6):
            t = sbp.tile([128, NCH[i], NLOC * B], FP, tag=f"tlT{i}")
            nc.sync.dma_start(out=t[:], in_=tlT[i][:].rearrange(
                "p (c n) -> p c n", c=NCH[i]))
            tl_sb.append(t)
            w = sbp.tile([128, NCH[i], OUT_F], FP, tag=f"wr{i}")
            nc.sync.dma_start(out=w[:], in_=wrs[i][:].rearrange(
                "p (c o) -> p c o", c=NCH[i]))
            wr_sb.append(w)
        t5_sb = sbp.tile([IN_F, NLOC, B], FP16)
        nc.sync.dma_start(out=t5_sb[:], in_=t5h[:])
        wbig_sb = sbp.tile([IN_F, 16], FP16)
        nc.sync.dma_start(out=wbig_sb[:], in_=wbig[:])
        abig_sb = sbp.tile([OUT_F, 16], FP16)
        nc.sync.dma_start(out=abig_sb[:], in_=abig[:])
        bsel_sb = sbp.tile([OUT_F, 16], FP16)
        nc.sync.dma_start(out=bsel_sb[:], in_=bsel[:])
        biasu_sb = sbp.tile([OUT_F, DEG], FP16)
        nc.sync.dma_start(out=biasu_sb[:], in_=biasu[:])
        oneh = sbp.tile([OUT_F, 1], FP)
        nc.vector.memset(oneh[:], 0.5)

        # ---- root aggregation: rootT fp16 (3, nl, b)
        proot = psp.tile([OUT_F, NLOC * B], FP, tag="proot")
        steps = [(i, c) for i in range(6) for c in range(NCH[i])]
        for si, (i, c) in enumerate(steps):
            nc.tensor.matmul(out=proot[:],
                             lhsT=wr_sb[i][:, c, :],
                             rhs=tl_sb[i][:, c, :],
                             start=(si == 0), stop=(si == len(steps) - 1))
        rootT = sbp.tile([OUT_F, NLOC, B], FP16)
        nc.scalar.activation(out=rootT[:],
                             in_=proot[:].rearrange("p (nl b) -> p nl b",
                                                    nl=NLOC),
                             func=AF.Copy)

        # ---- per-node: branch einsum (fp16) + leaky -> branchT fp16
        branchT = sbp.tile([IN_F, NLOC, B, 2, 32], FP16)
        xout_sb = sbp.tile([16, NLOC, B, DEG], FP)
        xx2_sb = sbp.tile([1, NLOC, B, DEG], FP)
        xh3_sb = sbp.tile([3, NLOC, B, DEG], FP16)
        xxh2_sb = sbp.tile([1, NLOC, B, DEG], FP16)
        xl3_sb = sbp.tile([3, NLOC, B, DEG], FP16)
        xxl2_sb = sbp.tile([1, NLOC, B, DEG], FP16)
        for nl in range(NLOC):
            wbt = wbpool.tile([IN_F, DEG * IN_F], FP16, tag="wbt")
            nc.sync.dma_start(out=wbt[:], in_=wb[nl])
            for g in range(2):
                pb = pbp.tile([IN_F, 32, B], FP, tag="pbranch")
                for dl in range(32):
                    d = g * 32 + dl
                    nc.tensor.matmul(out=pb[:, dl, :],
                                     lhsT=wbt[:, d * 128:(d + 1) * 128],
                                     rhs=t5_sb[:, nl, :],
                                     start=True, stop=True)
                # transpose-copy PSUM -> SBUF fp16 (b-major), then leaky
                cg = sbp.tile([IN_F, B, 32], FP16, tag="cg")
                nc.vector.tensor_copy(
                    out=cg[:].rearrange("p b dl -> p dl b"), in_=pb[:])
                nc.vector.scalar_tensor_tensor(
                    out=branchT[:, nl, :, g, :], in0=cg[:], scalar=0.2,
                    in1=cg[:], op0=ALU.mult, op1=ALU.max)
            # rows [x, y, u] for this node's points, 2 chunks of (8b x 64d)
            for h in range(2):
                pxo = pxp.tile([16, 8, DEG], FP, tag="pxo")
                nc.tensor.matmul(
                    out=pxo[:],
                    lhsT=wbig_sb[:],
                    rhs=branchT[:, nl, 8 * h:8 * h + 8, :, :].rearrange(
                        "p b g dl -> p (b g dl)"),
                    start=True, stop=False)
                nc.tensor.matmul(
                    out=pxo[:],
                    lhsT=abig_sb[:],
                    rhs=rootT[:, nl, 8 * h:8 * h + 8].unsqueeze(2)
                        .to_broadcast([OUT_F, 8, DEG]),
                    start=False, stop=False)
                nc.tensor.matmul(
                    out=pxo[:],
                    lhsT=bsel_sb[:],
                    rhs=biasu_sb[:].unsqueeze(1)
                        .to_broadcast([OUT_F, 8, DEG]),
                    start=False, stop=True)
                xo = xout_sb[:, nl, 8 * h:8 * h + 8, :]
                nc.scalar.activation(out=xo, in_=pxo[:], func=AF.Copy)
                # xx2 = 0.5 * sum x_c^2
                sq = sbp.tile([OUT_F, 8 * DEG], FP, tag="sq")
                nc.vector.scalar_tensor_tensor(
                    out=sq[:],
                    in0=xo[0:3].rearrange("p b d -> p (b d)"), scalar=0.0,
                    in1=xo[0:3].rearrange("p b d -> p (b d)"),
                    op0=ALU.bypass, op1=ALU.mult)
                pxx = psp.tile([1, 8 * DEG], FP, tag="pxx")
                nc.tensor.matmul(out=pxx[:], lhsT=oneh[:], rhs=sq[:],
                                 start=True, stop=True)
                nc.vector.tensor_copy(
                    out=xx2_sb[:, nl, 8 * h:8 * h + 8, :].rearrange(
                        "p b d -> p (b d)"), in_=pxx[:])
            # fp16 split of x and xx2 for this node (partition-0 aligned ops)
            nc.scalar.activation(out=xh3_sb[:, nl], in_=xout_sb[0:3, nl],
                                 func=AF.Copy)
            nc.scalar.activation(out=xxh2_sb[:, nl], in_=xx2_sb[:, nl],
                                 func=AF.Copy)
            nc.vector.scalar_tensor_tensor(
                out=xl3_sb[:, nl], in0=xout_sb[0:3, nl], scalar=0.0,
                in1=xh3_sb[:, nl], op0=ALU.bypass, op1=ALU.subtract)
            nc.vector.scalar_tensor_tensor(
                out=xxl2_sb[:, nl], in0=xx2_sb[:, nl], scalar=0.0,
                in1=xxh2_sb[:, nl], op0=ALU.bypass, op1=ALU.subtract)
            # per-node output stores (overlap with next node's wb load)
            CW = B * DEG
            nc.sync.dma_start(
                out=xout[0:3, nl * CW:(nl + 1) * CW],
                in_=xout_sb[0:3, nl].rearrange("p b d -> p (b d)"))
            nc.sync.dma_start(
                out=xout[4:16, nl * CW:(nl + 1) * CW],
                in_=xout_sb[4:16, nl].rearrange("p b d -> p (b d)"))
            nc.sync.dma_start(
                out=xout[3:4, nl * CW:(nl + 1) * CW],
                in_=xx2_sb[:, nl].rearrange("p b d -> p (b d)"))
            nc.sync.dma_start(
                out=xout16[0:3, nl * CW:(nl + 1) * CW],
                in_=xh3_sb[:, nl].rearrange("p b d -> p (b d)"))
            nc.sync.dma_start(
                out=xout16[3:4, nl * CW:(nl + 1) * CW],
                in_=xxh2_sb[:, nl].rearrange("p b d -> p (b d)"))
            nc.sync.dma_start(
                out=xout16[4:7, nl * CW:(nl + 1) * CW],
                in_=xl3_sb[:, nl].rearrange("p b d -> p (b d)"))
            nc.sync.dma_start(
                out=xout16[7:8, nl * CW:(nl + 1) * CW],
                in_=xxl2_sb[:, nl].rearrange("p b d -> p (b d)"))


    return nc


# --------------------------------------------------------------------------
# Stage B
# --------------------------------------------------------------------------
DEBUG_B = False
NF3 = 256   # level-3 fold slots


def build_stage_b():
    nc = bacc.Bacc(None)
    # k rows: 0-2 xh_j, 3-5 xl_j, 6-8 xh_j, 9 xxh2_j, 10 xxl2_j, 11 = -1
    vall = nc.declare_dram_parameter("vall", [12, B * N], FP16,
                                     isOutput=False)
    # k rows: 0-2 xh_n, 3-5 xh_n, 6-8 xl_n, 9-10 = -1, 11 xxh2_n
    uvl = nc.declare_dram_parameter("uvl", [12, NP_CORE], FP16,
                                    isOutput=False)
    rowdat = nc.declare_dram_parameter("rowdat", [128, 32 * 8], FP,
                                       isOutput=False)
    # ptab8[b][s, (u*4+q)*8 + c] = point j = s + 256u + 512q, vals
    # [x0, x1, x2, xx2, y0, y1, y2, 0]
    ptabs = [nc.declare_dram_parameter(f"ptab{b}", [NF3, 64], FP,
                                       isOutput=False) for b in range(B)]
    outc = nc.declare_dram_parameter("outc", [B, RLOC, OUT_F], FP,
                                     isOutput=True)
    if DEBUG_B:
        dbg_m3 = nc.declare_dram_parameter("dbg_m3", [128, NF3], FP16,
                                           isOutput=True)
        dbg_idx = nc.declare_dram_parameter("dbg_idx", [128, K], mybir.dt.uint16,
                                            isOutput=True)
        dbg_gth = nc.declare_dram_parameter("dbg_gth", [128, 8 * 64], FP,
                                            isOutput=True)
        dbg_pdc = nc.declare_dram_parameter("dbg_pdc", [128, K * 8], FP,
                                            isOutput=True)
        dbg_fr = nc.declare_dram_parameter("dbg_fr", [128, 32 * OUT_F], FP,
                                           isOutput=True)

    from concourse import library_config
    U16 = mybir.dt.uint16
    I16 = mybir.dt.int16

    with tile.TileContext(nc) as tc, ExitStack() as ctx:
        sbp = ctx.enter_context(tc.tile_pool(name="sbuf", bufs=1))
        lop = ctx.enter_context(tc.tile_pool(name="loop", bufs=4))
        bp = ctx.enter_context(tc.tile_pool(name="bloop", bufs=3))
        dramp = ctx.enter_context(tc.tile_pool(name="dram", bufs=2,
                                               space="DRAM"))
        pspd = ctx.enter_context(tc.tile_pool(name="pspd", bufs=2,
                                              space="PSUM"))

        nc.gpsimd.load_library(library_config.mlp)
        vall_sb = sbp.tile([12, B, N], FP16)
        nc.sync.dma_start(out=vall_sb[:],
                          in_=vall[:].rearrange("p (b n) -> p b n", b=B))
        uvl_sb = sbp.tile([12, NP_CORE], FP16)
        nc.sync.dma_start(out=uvl_sb[:], in_=uvl[:])
        rd_sb = sbp.tile([128, 32, 8], FP)
        nc.sync.dma_start(out=rd_sb[:],
                          in_=rowdat[:].rearrange("p (t v) -> p t v", t=32))
        final_raw = sbp.tile([128, 32, OUT_F], FP)

        for b in range(B):
            idx2 = bp.tile([128, 2, K], U16, tag="idx2")
            for m in range(2):
                t = b * 2 + m
                # ---- pd matmul (K=12 fp16 split-compensated)
                ppd = pspd.tile([128, N], FP, tag="ppd")
                for ch in range(4):
                    nc.tensor.matmul(
                        out=ppd[:, ch * NF:(ch + 1) * NF],
                        lhsT=uvl_sb[:, t * 128:(t + 1) * 128],
                        rhs=vall_sb[:, b, ch * NF:(ch + 1) * NF],
                        start=True, stop=True)
                # ---- folds 2048 -> 256:
                # m3[s] = max_{u,q} pd[s + 256u + 512q]
                call = lop.tile([128, 3, NF], FP16, tag="call")
                nc.scalar.activation(
                    out=call[:].rearrange("p c s -> p (c s)"),
                    in_=ppd[:, NF:2048], func=AF.Copy)
                m1 = lop.tile([128, 2, NF], FP16, tag="m1")
                nc.vector.tensor_tensor(
                    out=m1[:, 0, :], in0=ppd[:, 0:NF],
                    in1=call[:, 1, :], op=ALU.max)
                nc.vector.tensor_tensor(
                    out=m1[:, 1, :], in0=call[:, 0, :],
                    in1=call[:, 2, :], op=ALU.max)
                m2 = lop.tile([128, NF], FP16, tag="m2")
                nc.vector.tensor_tensor(
                    out=m2[:], in0=m1[:, 0, :], in1=m1[:, 1, :], op=ALU.max)
                m3 = lop.tile([128, NF3], FP16, tag="m3")
                nc.vector.tensor_tensor(
                    out=m3[:], in0=m2[:, 0:NF3], in1=m2[:, NF3:NF],
                    op=ALU.max)
                # ---- top-8 slots
                top8 = lop.tile([128, K], FP16, tag="top8")
                nc.vector.max(out=top8[:], in_=m3[:])
                nc.vector.max_index(out=idx2[:, m, :], in_max=top8[:],
                                    in_values=m3[:])
                if DEBUG_B and t == 0:
                    nc.sync.dma_start(out=dbg_m3[:], in_=m3[:])

            # ---- per-tile 1024-idx gathers (list elem i = k*128 + p ->
            # wrapped [i%16, i//16]: idxs16[r, k*8+a] = idx2[16a+r, m, k])
            gth = bp.tile([128, 16, 64], FP, tag="gth")
            idxs16 = bp.tile([128, 2, 64], I16, tag="idxs16")
            for m in range(2):
                scr = dramp.tile([128, K], U16, tag=f"scr{m}")
                nc.sync.dma_start(out=scr[:], in_=idx2[:, m, :])
                nc.sync.dma_start(
                    out=idxs16[0:16, m, :].rearrange("r (k a) -> r k a", k=K),
                    in_=scr[:].rearrange("(a r) k -> r k a", r=16)
                        .bitcast(I16))
            scr2 = dramp.tile([16, 128], U16, tag="scr2")
            nc.sync.dma_start(
                out=scr2[:],
                in_=idxs16[0:16, :, :].rearrange("r m c -> r (m c)")
                    .bitcast(U16))
            nc.sync.dma_start(
                out=idxs16[:].rearrange("p m c -> p (m c)"),
                in_=scr2[:].rearrange("r c -> (r c)").unsqueeze(0)
                    .to_broadcast([8, 16 * 128]).bitcast(I16))
            for m in range(2):
                nc.gpsimd.dma_gather(
                    gth[:, m * 8:(m + 1) * 8, :], ptabs[b][:],
                    idxs16[:, m, :], 1024, 1024, 64)
            if DEBUG_B and b == 0:
                nc.sync.dma_start(
                    out=dbg_idx[:], in_=idx2[:, 0, :])
                nc.sync.dma_start(
                    out=dbg_gth[:],
                    in_=gth[:, 0:8, :].rearrange("p g v -> p (g v)"))

            for m in range(2):
                t = b * 2 + m
                gt = gth[:, m * 8:(m + 1) * 8, :].rearrange(
                    "p k (e c) -> p k e c", c=8)
                # ---- exact fp32 pd for the 64 candidates (Pool chains)
                p0 = lop.tile([128, K, 8], FP, tag="p0")
                nc.gpsimd.tensor_scalar(out=p0[:], in0=gt[:, :, :, 0],
                                        scalar1=rd_sb[:, t, 0:1],
                                        scalar2=None, op0=ALU.mult)
                p1 = lop.tile([128, K, 8], FP, tag="p1")
                nc.gpsimd.tensor_scalar(out=p1[:], in0=gt[:, :, :, 1],
                                        scalar1=rd_sb[:, t, 1:2],
                                        scalar2=None, op0=ALU.mult)
                nc.gpsimd.tensor_add(out=p0[:], in0=p0[:], in1=p1[:])
                nc.gpsimd.tensor_scalar(out=p1[:], in0=gt[:, :, :, 2],
                                        scalar1=rd_sb[:, t, 2:3],
                                        scalar2=None, op0=ALU.mult)
                nc.gpsimd.tensor_add(out=p0[:], in0=p0[:], in1=p1[:])
                pdc = lop.tile([128, K, 8], FP, tag="pdc")
                nc.gpsimd.tensor_sub(out=pdc[:], in0=p0[:],
                                     in1=gt[:, :, :, 3])
                # ---- threshold at 8th largest of the 64
                t8 = lop.tile([128, K], FP, tag="t8")
                nc.vector.max(out=t8[:],
                              in_=pdc[:].rearrange("p k e -> p (k e)"))
                r = lop.tile([128, K, 8], FP, tag="r")
                nc.scalar.activation(out=r[:], in_=pdc[:], func=AF.Relu,
                                     bias=t8[:, 7:8], scale=-1.0)
                # ---- masked max of y over the selected 8
                rb = lop.tile([128, K * 8], FP, tag="rb")
                nc.gpsimd.tensor_scalar(
                    out=rb[:], in0=r[:].rearrange("p k e -> p (k e)"),
                    scalar1=-BIG, scalar2=None, op0=ALU.mult)
                ys = lop.tile([128, OUT_F, K * 8], FP, tag="ys")
                nc.gpsimd.tensor_add(
                    out=ys[:],
                    in0=rb[:].unsqueeze(1).to_broadcast([128, OUT_F, K * 8]),
                    in1=gt[:, :, :, 4:7].rearrange("p k e v -> p v (k e)"))
                nc.vector.tensor_reduce(
                    out=final_raw[:, t, :], in_=ys[:],
                    axis=mybir.AxisListType.X, op=ALU.max)
                if DEBUG_B and t == 0:
                    nc.sync.dma_start(
                        out=dbg_pdc[:],
                        in_=pdc[:].rearrange("p k e -> p (k e)"))

        if DEBUG_B:
            nc.sync.dma_start(
                out=dbg_fr[:],
                in_=final_raw[:].rearrange("p t o -> p (t o)"))

        # ---- epilogue: += u, leaky, store
        nc.vector.scalar_tensor_tensor(
            out=final_raw[:], in0=final_raw[:], scalar=0.0,
            in1=rd_sb[:, :, 3:6], op0=ALU.bypass, op1=ALU.add)
        nc.vector.scalar_tensor_tensor(
            out=final_raw[:], in0=final_raw[:], scalar=0.2,
            in1=final_raw[:], op0=ALU.mult, op1=ALU.max)
        nc.sync.dma_start(
            out=outc[:].rearrange("b (m p) o -> p (b m) o", p=128),
            in_=final_raw[:])
    return nc


# --------------------------------------------------------------------------
# Host orchestration
# --------------------------------------------------------------------------
_CACHE = {}
LAST_RESULTS = {}


def _programs():
    if "a" not in _CACHE:
        nca = build_stage_a()
        nca.compile()
        ncb = build_stage_b()
        ncb.compile()
        _CACHE["a"] = nca
        _CACHE["b"] = ncb
    return _CACHE["a"], _CACHE["b"]


def _weight_folds(inputs):
    c1w = np.asarray(inputs["c1w"], np.float32)
    c1b = np.asarray(inputs["c1b"], np.float32)
    c2w = np.asarray(inputs["c2w"], np.float32)
    c2b = np.asarray(inputs["c2b"], np.float32)
    M1 = c1w[:, :3].T @ c2w.T                      # (3, 3)
    M2 = c1w[:, 3:].T @ c2w.T                      # (3, 3)
    zc = (c1b @ c2w.T + c2b).reshape(3)
    Wl = (np.asarray(inputs["Wl1"], np.float32)
          @ np.asarray(inputs["Wl2"], np.float32))  # (128, 3)
    wbig = np.zeros((IN_F, 16), np.float32)
    wbig[:, 0:3] = Wl
    wbig[:, 4:7] = Wl @ M1
    wbig[:, 8:11] = Wl @ (M2 - M1)
    abig = np.zeros((OUT_F, 16), np.float32)
    abig[:, 0:3] = np.eye(3, dtype=np.float32)
    abig[:, 4:7] = M1
    abig[:, 8:11] = M2 - M1
    bsel = np.zeros((OUT_F, 16), np.float32)
    bsel[:, 8:11] = np.eye(3, dtype=np.float32)
    biasd = np.asarray(inputs["bias"], np.float32).reshape(DEG, OUT_F)
    biasu = np.ascontiguousarray((biasd + zc.reshape(1, 3)).T)  # (3, 64)
    return wbig, abig, bsel, biasu


def _stage_a_inmaps(inputs):
    trees = [np.asarray(inputs[f"t{i}"], np.float32) for i in range(6)]
    wrs = [np.asarray(inputs[f"Wr{i}"], np.float32) for i in range(6)]
    wbf = np.asarray(inputs["W_branch"], np.float32).astype(np.float16)
    wbig, abig, bsel, biasu = _weight_folds(inputs)
    t5 = trees[5]
    in_maps = []
    for c in range(NCORES):
        m = {}
        nodes = [NLOC * c + j for j in range(NLOC)]
        for i in range(6):
            f = FEATS[i]
            nch = NCH[i]
            rows = [n * SIZES[i] // NODE for n in nodes]
            sl = trees[i][:, rows, :].transpose(2, 1, 0).reshape(f, NLOC * B)
            slp = np.zeros((nch * 128, NLOC * B), np.float32)
            slp[:f] = sl
            m[f"tlT{i}"] = np.ascontiguousarray(
                slp.reshape(nch, 128, NLOC * B).transpose(1, 0, 2)
                .reshape(128, nch * NLOC * B))
            wp = np.zeros((nch * 128, OUT_F), np.float32)
            wp[:f] = wrs[i]
            m[f"wr{i}"] = np.ascontiguousarray(
                wp.reshape(nch, 128, OUT_F).transpose(1, 0, 2)
                .reshape(128, nch * OUT_F))
        m["wb"] = np.ascontiguousarray(wbf[nodes])
        m["t5h"] = np.ascontiguousarray(
            t5[:, nodes, :].transpose(2, 1, 0)).astype(np.float16)
        m["wbig"] = wbig.astype(np.float16)
        m["abig"] = abig.astype(np.float16)
        m["bsel"] = bsel.astype(np.float16)
        m["biasu"] = biasu.astype(np.float16)
        in_maps.append(m)
    return in_maps


def _stage_b_inmaps(inputs, xouts, xout16s):
    # xouts: per-core (16, 4096) fp32; xout16s: per-core (8, 4096) fp16
    # cols (nl, b, d); global j = c*256 + nl*64 + d
    xs = np.stack([np.asarray(x).reshape(16, NLOC, B, DEG) for x in xouts])
    hs = np.stack([np.asarray(x).reshape(8, NLOC, B, DEG) for x in xout16s])
    allp = xs.transpose(1, 3, 0, 2, 4).reshape(16, B, N)     # fp32
    allh = hs.transpose(1, 3, 0, 2, 4).reshape(8, B, N)      # fp16
    # vall rows: xh, xl, xh, xxh2, xxl2, -1
    vall = np.empty((12, B, N), np.float16)
    vall[0:3] = allh[0:3]
    vall[3:6] = allh[4:7]
    vall[6:9] = allh[0:3]
    vall[9] = allh[3]
    vall[10] = allh[7]
    vall[11] = -1.0
    vall = np.ascontiguousarray(vall.reshape(12, B * N))
    ptabs = {}
    for b in range(B):
        pt = np.zeros((N, 8), np.float32)
        pt[:, 0:7] = allp[0:7, b].T       # x(3), xx2, y(3)
        # ptab8[s, e=(u*4+q), :] = pt[s + 256u + 512q]
        p8 = pt.reshape(2, 4, 256, 8).transpose(2, 0, 1, 3)  # (s, u, q, 8)
        ptabs[f"ptab{b}"] = np.ascontiguousarray(p8.reshape(256, 64))
    in_maps = []
    for c in range(NCORES):
        own = xs[c].transpose(0, 2, 1, 3).reshape(16, B, RLOC)
        ownh = hs[c].transpose(0, 2, 1, 3).reshape(8, NP_CORE)
        u = np.empty((12, NP_CORE), np.float16)
        u[0:3] = ownh[0:3]
        u[3:6] = ownh[0:3]
        u[6:9] = ownh[4:7]
        u[9:11] = -1.0
        u[11] = ownh[3]
        rd = np.zeros((128, 32, 8), np.float32)
        o2 = own.reshape(16, B, 2, 128)
        rd[:, :, 0:3] = o2[0:3].transpose(3, 1, 2, 0).reshape(128, 32, 3)
        rd[:, :, 3:6] = o2[8:11].transpose(3, 1, 2, 0).reshape(128, 32, 3)
        m = {"vall": vall, "uvl": np.ascontiguousarray(u),
             "rowdat": np.ascontiguousarray(rd.reshape(128, 32 * 8))}
        m.update(ptabs)
        in_maps.append(m)
    return in_maps


def kernel(**inputs):
    nca, ncb = _programs()
    core_ids = list(range(NCORES))

    ra = run_bass_kernel_spmd(nca, _stage_a_inmaps(inputs), core_ids)
    LAST_RESULTS["a"] = ra
    xouts = [np.asarray(ra.results[c]["xout"]) for c in range(NCORES)]
    xout16s = [np.asarray(ra.results[c]["xout16"]) for c in range(NCORES)]

    rb = run_bass_kernel_spmd(ncb, _stage_b_inmaps(inputs, xouts, xout16s),
                              core_ids)
    LAST_RESULTS["b"] = rb
    out = np.empty((B, N, OUT_F), np.float32)
    for c in range(NCORES):
        out[:, c * RLOC:(c + 1) * RLOC, :] = rb.results[c]["outc"]
    return out
